# revision 19
# baseline (speedup 1.0000x reference)
"""Distributed Trainium2 kernel for batched multi-head self-attention with
positional bias.

Reference computation (per batch element b):
    qkv = x[b] @ w_qkv ; split into q,k,v ; heads of 64
    sim = (q * 64**-0.5) @ k^T + pos_bias          # [h, n, n]
    attn = softmax(sim, axis=-1)
    out[b] = (attn @ v).reshape(n, hidden) @ w_out

Sharding: pure data-parallel - core i computes batch element i (B == 8 ==
n_cores), no collectives.

Device algorithm (per core), designed to avoid all on-chip transposes:
  - host supplies xT = x[b].T, so projections produce Q^T,K^T ([d, n]) and V
    ([n, d]) directly with natural-layout matmuls.
  - attention is computed transposed: St[j,i] = sum_d K^T[d,j] Q^T[d,i];
    softmax over j is handled via exp (ScalarE) * exp(bias^T) (host
    precomputed, fp16, prepacked per-tile) and a ones-block in the AV
    matmul's stationary operand, which makes PSUM rows 0:64 the softmax
    denominators.
  - U''[64:128] * 1/U''[0:64] gives the normalized per-head context, already
    in the [hidden, n] layout the output projection needs as lhsT.

Scheduling (v2): fine-grained input DMAs split across the sync and scalar
HW-DGE queues so the first projections and first bias tiles land ~5us
earlier; projection matmuls are spread ~2 per attention step as PE gap
filler (the exp->mul chain latency otherwise stalls the AV matmuls); PSUM
drains alternate ScalarE/VectorE.
"""

import numpy as np

B, N, D = 8, 1024, 512
H, DH = 8, 64
SCALE = DH**-0.5
NCORES = 8
KT = D // 128  # 4 k-tiles over model dim / hidden dim
NJT = N // 128  # 8 j-tiles
IB = 512
NIB = N // IB  # 2 i-blocks
NWARM = 12

_CACHE = {}


def _build_graph(sim=False):
    import concourse.bass as bass
    import concourse.mybir as mybir
    from concourse import tile

    f32 = mybir.dt.float32
    f16 = mybir.dt.float16
    Exp = mybir.ActivationFunctionType.Exp

    import concourse.bacc as bacc

    # target_bir_lowering=False: bass/bacc lower to per-engine streams with
    # standalone waits itself; walrus's sync structs hold few waits and
    # reject Tile-generated multi-wait instructions otherwise.
    nc = bacc.Bacc(None, target_bir_lowering=False, debug=False)
    xT = nc.declare_dram_parameter("xT", [D, N], f16, isOutput=False)
    wqkv = nc.declare_dram_parameter("wqkv", [D, 3 * D], f16, isOutput=False)
    wout = nc.declare_dram_parameter("wout", [D, D], f16, isOutput=False)
    # host-prepacked exp(bias^T) tiles: ebt[t, ib, jt] = [128 j, he-i | ho-i]
    ebt = nc.declare_dram_parameter(
        "ebt", [KT, NIB, NJT, 128, 2 * IB], f16, isOutput=False
    )
    out = nc.declare_dram_parameter("out", [N, D], f32, isOutput=True)

    with tile.TileContext(nc) as tc:
        with (
            tc.tile_pool(name="const", bufs=1) as cpool,
            tc.tile_pool(name="mm_ps", bufs=4, space="PSUM") as mm_ps,
            tc.tile_pool(name="st_ps", bufs=2, space="PSUM") as st_ps,
            tc.tile_pool(name="stream", bufs=4) as stream,
            tc.tile_pool(name="osb", bufs=2) as opool,
        ):
            # ---- Phase 0: resident allocation + fine-grained loads ----
            # sync queue: xT k-tiles, then w q-cols, then w k-cols, then the
            # steady eb stream.  scalar queue (idle engine at start): w
            # v-cols, the first 4 eb tiles, and wout - all off the critical
            # path of the first Q/K projections.
            w_sb = [
                cpool.tile([128, 3 * D], f16, tag=f"w{k}", name=f"w{k}")
                for k in range(KT)
            ]
            xT_sb = [
                cpool.tile([128, N], f16, tag=f"xt{k}", name=f"xt{k}")
                for k in range(KT)
            ]
            wout_sb = [
                cpool.tile([128, D], f16, tag=f"wo{k}", name=f"wo{k}")
                for k in range(KT)
            ]
            for k in range(KT):
                nc.sync.dma_start(xT_sb[k][:], xT[k * 128 : (k + 1) * 128, :])
            # only the t=0 column slices of w_q / w_k gate the first Q/K
            # projections - load them first (critical startup set = 1.25MB),
            # the t=1..3 slices follow and are needed ~10us later
            for c0, c1 in ((0, 128), (D, D + 128), (128, D), (D + 128, 2 * D)):
                for k in range(KT):
                    nc.sync.dma_start(
                        w_sb[k][:, c0:c1], wqkv[k * 128 : (k + 1) * 128, c0:c1]
                    )
            # v-cols first on the scalar queue: v_proj(0..2) run right after
            # the three upfront q/k projection groups
            for k in range(KT):
                nc.scalar.dma_start(
                    w_sb[k][:, 2 * D : 3 * D],
                    wqkv[k * 128 : (k + 1) * 128, 2 * D : 3 * D],
                )

            # V1: per jt a [128, H*128] tensor holding, per head, the AV
            # stationary operand [ones | v_h] (ones via memset, v written by
            # the V projection).
            V1_sb = []
            for jt in range(NJT):
                v1 = cpool.tile([128, H * 128], f16, tag=f"v1_{jt}", name=f"v1_{jt}")
                nc.gpsimd.memset(v1[:], 1.0)
                V1_sb.append(v1)

            QT_sb = [cpool.tile([128, N], f16, tag=f"qt{t}", name=f"qt{t}") for t in range(KT)]
            KT_sb = [cpool.tile([128, N], f16, tag=f"kt{t}", name=f"kt{t}") for t in range(KT)]
            Utn_sb = [
                [
                    cpool.tile([128, IB], f16, tag=f"ut{t}_{ib}", name=f"ut{t}_{ib}")
                    for ib in range(NIB)
                ]
                for t in range(KT)
            ]

            # eb DMA launcher: first block's tiles ride the scalar queue
            # (issued during startup), everything else the sync queue.
            eb_count = [0]

            def load_eb(t, ib, jt):
                eb = stream.tile([128, 2 * IB], f16, tag="eb", bufs=6)
                eng = nc.scalar if eb_count[0] < 4 else nc.sync
                eb_count[0] += 1
                eng.dma_start(eb[:], ebt[t, ib, jt, :, :])
                return eb

            # ---- Phase 0b: PE warm-up + ACT exp-table preload during the
            # input-DMA window. Dummy matmuls keep the PE HAM busy while
            # xT/w DMAs land, so real matmuls start at 2.4 GHz.
            dumA = cpool.tile([128, 128], f16, tag="dumA", name="dumA")
            dumB = cpool.tile([128, 512], f16, tag="dumB", name="dumB")
            dumE = cpool.tile([128, 64], f16, tag="dumE", name="dumE")
            nc.gpsimd.memset(dumA[:], 0.0)
            nc.gpsimd.memset(dumB[:], 0.0)
            # exp-table preload so the first real exp doesn't pay the
            # ~1.3us ACT_TABLE_LOAD
            nc.scalar.activation(dumE[:], dumB[:, 0:64], Exp)
            for i in range(NWARM):
                wps = mm_ps.tile([128, IB], f32, tag="ps", name=f"warm{i}")
                nc.tensor.matmul(wps[:], dumA[:], dumB[:], start=True, stop=True)

            copy_count = [0]

            def psum_copy(dst, src):
                # PSUM->SBUF drains alternate ScalarE/VectorE so neither
                # pacing engine absorbs them all.
                copy_count[0] += 1
                if copy_count[0] % 2:
                    nc.scalar.copy(dst, src)
                else:
                    nc.vector.tensor_copy(dst, src)

            def qk_proj_mms(t, ib, which):
                # one Q^T (which=0) or K^T (which=1) projection group as a
                # list of single-matmul closures + final copy closure, so the
                # scheduler can spread them across attention steps.
                dst = QT_sb if which == 0 else KT_sb
                col0 = 0 if which == 0 else D
                ps_box = []

                def mm(k):
                    def go():
                        if k == 0:
                            ps_box.append(
                                mm_ps.tile([128, IB], f32, tag="ps", name=f"qkp{t}{ib}{which}")
                            )
                        nc.tensor.matmul(
                            ps_box[0][:],
                            w_sb[k][:, col0 + t * 128 : col0 + (t + 1) * 128],
                            xT_sb[k][:, ib * IB : (ib + 1) * IB],
                            start=(k == 0),
                            stop=(k == KT - 1),
                        )
                        if k == KT - 1:
                            psum_copy(dst[t][:, ib * IB : (ib + 1) * IB], ps_box[0][:])
                    return go

                return [mm(k) for k in range(KT)]

            def v_proj_mms(nt):
                ps_box = []

                def mm(k):
                    def go():
                        if k == 0:
                            ps_box.append(
                                mm_ps.tile([128, D], f32, tag="ps", name=f"vps{nt}")
                            )
                        nc.tensor.matmul(
                            ps_box[0][:],
                            xT_sb[k][:, nt * 128 : (nt + 1) * 128],
                            w_sb[k][:, 2 * D : 3 * D],
                            start=(k == 0),
                            stop=(k == KT - 1),
                        )
                        if k == KT - 1:
                            nc.vector.tensor_copy(
                                V1_sb[nt].rearrange("p (h c) -> p h c", h=H)[:, :, DH : 2 * DH],
                                ps_box[0].rearrange("p (h c) -> p h c", h=H)[:, :, :],
                            )
                    return go

                return [mm(k) for k in range(KT)]

            def out_proj_mms(nt, pool=None, tag="ps"):
                ps_box = []
                pool_ = pool if pool is not None else mm_ps

                def mm(k):
                    def go():
                        if k == 0:
                            ps_box.append(
                                pool_.tile([128, D], f32, tag=tag, name=f"ops{nt}")
                            )
                        nc.tensor.matmul(
                            ps_box[0][:],
                            Utn_sb[k][nt // 4][:, (nt % 4) * 128 : (nt % 4 + 1) * 128],
                            wout_sb[k][:],
                            start=(k == 0),
                            stop=(k == KT - 1),
                        )
                        if k == KT - 1:
                            osb = opool.tile([128, D], f32, tag="osb", name=f"osb{nt}")
                            psum_copy(osb[:], ps_box[0][:])
                            nc.sync.dma_start(out[nt * 128 : (nt + 1) * 128, :], osb[:])
                    return go

                return [mm(k) for k in range(KT)]

            def run_group(mms):
                for fn in mms:
                    fn()

            # Minimal pre-attention work: only what block (ib0, t0) needs up
            # front. Everything else is interleaved at scheduled (block, jt)
            # slots, ~2 matmuls per slot, so it rides in the PE's idle time
            # while ScalarE streams exps.
            run_group(qk_proj_mms(0, 0, 0))  # QT[t0] i-cols 0:512
            run_group(qk_proj_mms(0, 0, 1))  # KT[t0] j-cols 0:512
            run_group(qk_proj_mms(0, 1, 1))  # KT[t0] j-cols 512:1024
            run_group(v_proj_mms(0))
            run_group(v_proj_mms(1))
            run_group(v_proj_mms(2))

            blocks = [(ib, t) for ib in range(NIB) for t in range(KT)]
            # tasks[(bi, jt)] = list of closures (individual matmuls/copies)
            tasks = {}

            def sched(bi, jt, mms, per_slot=2):
                # spread a group's matmuls over consecutive jt slots,
                # per_slot per slot starting at (bi, jt).  NOTE: a group
                # consumed by the next block's pre-issued QK must fully land
                # by slot (bi, 6) - slot (bi, 7)'s tasks are emitted after
                # the pre_qk for block bi+1.
                for i, fn in enumerate(mms):
                    slot = jt + i // per_slot
                    b2, j2 = bi + slot // NJT, slot % NJT
                    tasks.setdefault((b2, j2), []).append(fn)

            # remaining V projections in block 0 (PE has slack there while
            # the exp chain ramps): v_proj(nt) complete before AV(jt=nt)
            for nt in range(3, NJT):
                sched(0, nt - 3, v_proj_mms(nt), per_slot=4)
            # Q^T/K^T ib0 for pair tn, finishing before block tn's first QK
            # (pre-issued at (tn-1, jt=7))
            sched(0, 5, qk_proj_mms(1, 0, 0), per_slot=4)
            sched(0, 6, qk_proj_mms(1, 0, 1), per_slot=4)
            for bi, tn in ((1, 2), (2, 3)):
                sched(bi, 3, qk_proj_mms(tn, 0, 0))
                sched(bi, 4, qk_proj_mms(tn, 0, 1))
            # K^T j-cols 512:1024 of pair tn, needed from block tn's jt=4
            for bi, tn in ((0, 1), (1, 2), (2, 3)):
                sched(bi + 1, 1, qk_proj_mms(tn, 1, 1))
            for bi in range(4):
                # QT i-cols 512:1024 of pair bi, needed from block 4+bi,
                # whose first QK pre-issues at (3+bi, jt=7)
                sched(bi + 1, 5, qk_proj_mms(bi, 1, 0))
            # wout loads on the scalar queue after the first eb tiles (slot
            # (0,5) -> trigger lands behind eb0-3), well before block 5
            def load_wout():
                for k in range(KT):
                    nc.scalar.dma_start(
                        wout_sb[k][:], wout[k * 128 : (k + 1) * 128, :]
                    )

            tasks.setdefault((0, 5), []).insert(0, load_wout)
            # out projections for the ib=0 half: Utn[*][0] ready after block
            # 3's norms (flushed at block 4, jt=1)
            for nt in range(4):
                sched(5 + nt // 2, 2 + 3 * (nt % 2), out_proj_mms(nt))
            # ib=1 half, k-tiles 0..2: Utn[0..2][1] are ready once block 6's
            # norms flush at (7,1) - pre-accumulate nt=4,5 during block 7 so
            # only their k=3 matmul (plus nt=6,7) remains after the final
            # norm.  mm_ps has exactly 2 free slots alongside block 7's ups.
            tail_pre = {nt: out_proj_mms(nt) for nt in (4, 5)}
            for nt in (6, 7):
                # nt=6,7 accumulate in st_ps slots, which free up as block
                # 7's last exps drain - their k=0..2 matmuls fill the PE's
                # tail window while the final norms run
                tail_pre[nt] = out_proj_mms(nt, pool=st_ps, tag="st")
            for i, nt in enumerate((4, 5)):
                sched(7, 2 + 2 * i, tail_pre[nt][:KT - 1], per_slot=2)

            # ---- Phase 2: attention (transposed), even/odd heads paired ----
            # The two heads of pair t sit at partitions 0:64 / 64:128 of
            # QT_sb[t]/KT_sb[t]. One exp / one bias-multiply covers both.
            pending_norms = []

            def flush_norms():
                while pending_norms:
                    pending_norms.pop(0)()

            def make_qk(t, ib):
                def qk(jt):
                    st = st_ps.tile(
                        [128, 2 * IB], f32, bufs=2, tag="st", name=f"st{t}{ib}{jt}"
                    )
                    nc.tensor.matmul(
                        st[:, 0:IB],
                        KT_sb[t][0:64, jt * 128 : (jt + 1) * 128],
                        QT_sb[t][0:64, ib * IB : (ib + 1) * IB],
                        start=True,
                        stop=True,
                    )
                    nc.tensor.matmul(
                        st[:, IB : 2 * IB],
                        KT_sb[t][64:128, jt * 128 : (jt + 1) * 128],
                        QT_sb[t][64:128, ib * IB : (ib + 1) * IB],
                        start=True,
                        stop=True,
                    )
                    return st
                return qk

            pre_qk = None
            for bi, (ib, t) in enumerate(blocks):
                he, ho = 2 * t, 2 * t + 1
                ups_e = mm_ps.tile([128, IB], f32, tag="ps", name=f"upse{t}{ib}")
                ups_o = mm_ps.tile([128, IB], f32, tag="ps", name=f"upso{t}{ib}")
                qk = make_qk(t, ib)

                def make_av(jt, et, ups_e=ups_e, ups_o=ups_o, he=he, ho=ho):
                    def go():
                        nc.tensor.matmul(
                            ups_e[:],
                            V1_sb[jt][:, he * 128 : (he + 1) * 128],
                            et[:, 0:IB],
                            start=(jt == 0),
                            stop=(jt == NJT - 1),
                        )
                        nc.tensor.matmul(
                            ups_o[:],
                            V1_sb[jt][:, ho * 128 : (ho + 1) * 128],
                            et[:, IB : 2 * IB],
                            start=(jt == 0),
                            stop=(jt == NJT - 1),
                        )
                    return go

                # software pipeline: QK(jt+1) issues on PE before AV(jt), and
                # AV(jt) is deferred a full slot (emitted at jt+1, behind the
                # filler tasks) - the PE queue is in-order, so this gives the
                # QK->exp->mul chain ~2 steps of latency budget before an
                # unready et can block the PE.
                sts = [pre_qk] if pre_qk is not None else [qk(0)]
                pre_qk = None
                pend_av = None
                for jt in range(NJT):
                    if jt + 1 < NJT:
                        sts.append(qk(jt + 1))
                    elif bi + 1 < len(blocks):
                        nib, nt_ = blocks[bi + 1]
                        pre_qk = make_qk(nt_, nib)(0)
                    st = sts[jt]
                    eb = load_eb(t, ib, jt)
                    et0 = stream.tile([128, 2 * IB], f16, tag="et0", bufs=5)
                    nc.scalar.activation(et0[:], st[:], Exp)
                    et = stream.tile([128, 2 * IB], f16, tag="et", bufs=5)
                    nc.vector.tensor_mul(et[:], et0[:], eb[:])
                    if jt in (1, 2) and pending_norms:
                        # one half of the previous block's norms per slot,
                        # deferred + split so the DVE never sees a >1.5us
                        # burst between this block's et-multiplies
                        pending_norms.pop(0)()
                    for fn in tasks.get((bi, jt), ()):
                        fn()
                    if pend_av is not None:
                        pend_av()
                    pend_av = make_av(jt, et)
                if bi + 1 < len(blocks):
                    pend_av()
                else:
                    # last block: slot the nt=6,7 out-projection partials
                    # around the final AV so the PE tail window stays full
                    for fn in tail_pre[6][: KT - 1]:
                        fn()
                    pend_av()
                    for fn in tail_pre[7][: KT - 1]:
                        fn()

                def make_norm(po, ups, t=t, ib=ib):
                    def go():
                        rb = stream.tile(
                            [64, IB], f32, tag="rb", name=f"rb{t}{ib}{po}"
                        )
                        nc.vector.reciprocal_approx_fast(rb[:, :], ups[0:64, :])
                        nc.vector.tensor_mul(
                            Utn_sb[t][ib][po : po + 64, :],
                            ups[64:128, :],
                            rb[:, :],
                        )
                    return go

                pending_norms.append(make_norm(0, ups_e))
                pending_norms.append(make_norm(64, ups_o))
            flush_norms()

            # ---- Phase 3: final k-tile of each remaining output projection ----
            for nt in (4, 5, 6, 7):
                run_group(tail_pre[nt][KT - 1 :])

    return nc


def _get_graph():
    if "nc" not in _CACHE:
        nc = _build_graph()
        nc.compile()
        _CACHE["nc"] = nc
    return _CACHE["nc"]


def _prep_inputs(x, pos_bias, w_qkv, w_out):
    x = np.asarray(x, dtype=np.float32)
    pos_bias = np.asarray(pos_bias, dtype=np.float32)
    w_qkv = np.asarray(w_qkv, dtype=np.float32)
    w_out = np.asarray(w_out, dtype=np.float32)

    wqkv_mod = w_qkv.copy()
    wqkv_mod[:, :D] *= SCALE
    wout16 = w_out.astype(np.float16)
    wqkv16 = wqkv_mod.astype(np.float16)
    # prepacked exp(bias^T) tiles: ebt[t, ib, jt] = [128 j, he-i | ho-i]
    ebt = np.exp(pos_bias.transpose(0, 2, 1)).astype(np.float16)  # [h, j, i]
    ebt4 = ebt.reshape(KT, 2, NJT, 128, NIB, IB)  # [t, par, jt, p, ib, i]
    ebt_tiles = np.ascontiguousarray(
        ebt4.transpose(0, 4, 2, 3, 1, 5).reshape(KT, NIB, NJT, 128, 2 * IB)
    )

    in_maps = []
    for b in range(NCORES):
        in_maps.append(
            {
                "xT": np.ascontiguousarray(x[b].T.astype(np.float16)),
                "wqkv": wqkv16,
                "wout": wout16,
                "ebt": ebt_tiles,
            }
        )
    return in_maps


def _run(x, pos_bias, w_qkv, w_out, trace=False):
    from concourse.bass_utils import run_bass_kernel_spmd

    nc = _get_graph()
    in_maps = _prep_inputs(x, pos_bias, w_qkv, w_out)
    res = run_bass_kernel_spmd(
        nc, in_maps, core_ids=list(range(NCORES)), trace=trace
    )
    outs = np.stack([np.asarray(res.results[b]["out"]) for b in range(NCORES)])
    return outs.astype(np.float32), res


def kernel(x, pos_bias, w_qkv, w_out):
    outs, _ = _run(x, pos_bias, w_qkv, w_out, trace=False)
    return outs


# revision 24
# speedup vs baseline: 1.0045x; 1.0045x over previous
"""Distributed Trainium2 kernel for batched multi-head self-attention with
positional bias.

Reference computation (per batch element b):
    qkv = x[b] @ w_qkv ; split into q,k,v ; heads of 64
    sim = (q * 64**-0.5) @ k^T + pos_bias          # [h, n, n]
    attn = softmax(sim, axis=-1)
    out[b] = (attn @ v).reshape(n, hidden) @ w_out

Sharding: pure data-parallel - core i computes batch element i (B == 8 ==
n_cores), no collectives.

Device algorithm (per core), designed to avoid all on-chip transposes:
  - host supplies xT = x[b].T, so projections produce Q^T,K^T ([d, n]) and V
    ([n, d]) directly with natural-layout matmuls.
  - attention is computed transposed: St[j,i] = sum_d K^T[d,j] Q^T[d,i];
    softmax over j is handled via exp (ScalarE) * exp(bias^T) (host
    precomputed, fp16, prepacked per-tile) and a ones-block in the AV
    matmul's stationary operand, which makes PSUM rows 0:64 the softmax
    denominators.
  - U''[64:128] * 1/U''[0:64] gives the normalized per-head context, already
    in the [hidden, n] layout the output projection needs as lhsT.

Scheduling (v2): fine-grained input DMAs split across the sync and scalar
HW-DGE queues so the first projections and first bias tiles land ~5us
earlier; projection matmuls are spread ~2 per attention step as PE gap
filler (the exp->mul chain latency otherwise stalls the AV matmuls); PSUM
drains alternate ScalarE/VectorE.
"""

import numpy as np

B, N, D = 8, 1024, 512
H, DH = 8, 64
SCALE = DH**-0.5
NCORES = 8
KT = D // 128  # 4 k-tiles over model dim / hidden dim
NJT = N // 128  # 8 j-tiles
IB = 512
NIB = N // IB  # 2 i-blocks
NWARM = 10

_CACHE = {}


def _build_graph(sim=False):
    import concourse.bass as bass
    import concourse.mybir as mybir
    from concourse import tile

    f32 = mybir.dt.float32
    f16 = mybir.dt.float16
    Exp = mybir.ActivationFunctionType.Exp

    import concourse.bacc as bacc

    # target_bir_lowering=False: bass/bacc lower to per-engine streams with
    # standalone waits itself; walrus's sync structs hold few waits and
    # reject Tile-generated multi-wait instructions otherwise.
    nc = bacc.Bacc(None, target_bir_lowering=False, debug=False)
    xT = nc.declare_dram_parameter("xT", [D, N], f16, isOutput=False)
    wqkv = nc.declare_dram_parameter("wqkv", [D, 3 * D], f16, isOutput=False)
    wout = nc.declare_dram_parameter("wout", [D, D], f16, isOutput=False)
    # host-prepacked exp(bias^T) tiles: ebt[t, ib, jt] = [128 j, he-i | ho-i]
    ebt = nc.declare_dram_parameter(
        "ebt", [KT, NIB, NJT, 128, 2 * IB], f16, isOutput=False
    )
    out = nc.declare_dram_parameter("out", [N, D], f32, isOutput=True)

    with tile.TileContext(nc) as tc:
        with (
            tc.tile_pool(name="const", bufs=1) as cpool,
            tc.tile_pool(name="mm_ps", bufs=4, space="PSUM") as mm_ps,
            tc.tile_pool(name="st_ps", bufs=2, space="PSUM") as st_ps,
            tc.tile_pool(name="stream", bufs=4) as stream,
            tc.tile_pool(name="osb", bufs=2) as opool,
        ):
            # ---- Phase 0: resident allocation + fine-grained loads ----
            # sync queue: xT k-tiles, then w q-cols, then w k-cols, then the
            # steady eb stream.  scalar queue (idle engine at start): w
            # v-cols, the first 4 eb tiles, and wout - all off the critical
            # path of the first Q/K projections.
            w_sb = [
                cpool.tile([128, 3 * D], f16, tag=f"w{k}", name=f"w{k}")
                for k in range(KT)
            ]
            xT_sb = [
                cpool.tile([128, N], f16, tag=f"xt{k}", name=f"xt{k}")
                for k in range(KT)
            ]
            wout_sb = [
                cpool.tile([128, D], f16, tag=f"wo{k}", name=f"wo{k}")
                for k in range(KT)
            ]
            for k in range(KT):
                nc.sync.dma_start(xT_sb[k][:], xT[k * 128 : (k + 1) * 128, :])
            for k in range(KT):
                nc.sync.dma_start(
                    w_sb[k][:, 0:D], wqkv[k * 128 : (k + 1) * 128, 0:D]
                )
            for k in range(KT):
                nc.sync.dma_start(
                    w_sb[k][:, D : 2 * D], wqkv[k * 128 : (k + 1) * 128, D : 2 * D]
                )
            # v-cols first on the scalar queue: v_proj(0..2) run right after
            # the three upfront q/k projection groups
            for k in range(KT):
                nc.scalar.dma_start(
                    w_sb[k][:, 2 * D : 3 * D],
                    wqkv[k * 128 : (k + 1) * 128, 2 * D : 3 * D],
                )

            # V1: per jt a [128, H*128] tensor holding, per head, the AV
            # stationary operand [ones | v_h] (ones via memset, v written by
            # the V projection).
            V1_sb = []
            for jt in range(NJT):
                v1 = cpool.tile([128, H * 128], f16, tag=f"v1_{jt}", name=f"v1_{jt}")
                nc.gpsimd.memset(v1[:], 1.0)
                V1_sb.append(v1)

            QT_sb = [cpool.tile([128, N], f16, tag=f"qt{t}", name=f"qt{t}") for t in range(KT)]
            KT_sb = [cpool.tile([128, N], f16, tag=f"kt{t}", name=f"kt{t}") for t in range(KT)]
            Utn_sb = [
                [
                    cpool.tile([128, IB], f16, tag=f"ut{t}_{ib}", name=f"ut{t}_{ib}")
                    for ib in range(NIB)
                ]
                for t in range(KT)
            ]

            # eb DMA launcher: first block's tiles ride the scalar queue
            # (issued during startup), everything else the sync queue.
            eb_count = [0]

            def load_eb(t, ib, jt):
                eb = stream.tile([128, 2 * IB], f16, tag="eb", bufs=6)
                eng = nc.scalar if eb_count[0] < 4 else nc.sync
                eb_count[0] += 1
                eng.dma_start(eb[:], ebt[t, ib, jt, :, :])
                return eb

            # ---- Phase 0b: PE warm-up + ACT exp-table preload during the
            # input-DMA window. Dummy matmuls keep the PE HAM busy while
            # xT/w DMAs land, so real matmuls start at 2.4 GHz.
            dumA = cpool.tile([128, 128], f16, tag="dumA", name="dumA")
            dumB = cpool.tile([128, 512], f16, tag="dumB", name="dumB")
            dumE = cpool.tile([128, 64], f16, tag="dumE", name="dumE")
            nc.gpsimd.memset(dumA[:], 0.0)
            nc.gpsimd.memset(dumB[:], 0.0)
            # exp-table preload so the first real exp doesn't pay the
            # ~1.3us ACT_TABLE_LOAD
            nc.scalar.activation(dumE[:], dumB[:, 0:64], Exp)
            for i in range(NWARM):
                wps = mm_ps.tile([128, IB], f32, tag="ps", name=f"warm{i}")
                nc.tensor.matmul(wps[:], dumA[:], dumB[:], start=True, stop=True)

            copy_count = [0]

            def psum_copy(dst, src):
                # PSUM->SBUF drains alternate ScalarE/VectorE so neither
                # pacing engine absorbs them all.
                copy_count[0] += 1
                if copy_count[0] % 2:
                    nc.scalar.copy(dst, src)
                else:
                    nc.vector.tensor_copy(dst, src)

            def qk_proj_mms(t, ib, which):
                # one Q^T (which=0) or K^T (which=1) projection group as a
                # list of single-matmul closures + final copy closure, so the
                # scheduler can spread them across attention steps.
                dst = QT_sb if which == 0 else KT_sb
                col0 = 0 if which == 0 else D
                ps_box = []

                def mm(k):
                    def go():
                        if k == 0:
                            ps_box.append(
                                mm_ps.tile([128, IB], f32, tag="ps", name=f"qkp{t}{ib}{which}")
                            )
                        nc.tensor.matmul(
                            ps_box[0][:],
                            w_sb[k][:, col0 + t * 128 : col0 + (t + 1) * 128],
                            xT_sb[k][:, ib * IB : (ib + 1) * IB],
                            start=(k == 0),
                            stop=(k == KT - 1),
                        )
                        if k == KT - 1:
                            psum_copy(dst[t][:, ib * IB : (ib + 1) * IB], ps_box[0][:])
                    return go

                return [mm(k) for k in range(KT)]

            def v_proj_mms(nt):
                ps_box = []

                def mm(k):
                    def go():
                        if k == 0:
                            ps_box.append(
                                mm_ps.tile([128, D], f32, tag="ps", name=f"vps{nt}")
                            )
                        nc.tensor.matmul(
                            ps_box[0][:],
                            xT_sb[k][:, nt * 128 : (nt + 1) * 128],
                            w_sb[k][:, 2 * D : 3 * D],
                            start=(k == 0),
                            stop=(k == KT - 1),
                        )
                        if k == KT - 1:
                            nc.vector.tensor_copy(
                                V1_sb[nt].rearrange("p (h c) -> p h c", h=H)[:, :, DH : 2 * DH],
                                ps_box[0].rearrange("p (h c) -> p h c", h=H)[:, :, :],
                            )
                    return go

                return [mm(k) for k in range(KT)]

            def out_proj_mms(nt, pool=None, tag="ps"):
                ps_box = []
                pool_ = pool if pool is not None else mm_ps

                def mm(k):
                    def go():
                        if k == 0:
                            ps_box.append(
                                pool_.tile([128, D], f32, tag=tag, name=f"ops{nt}")
                            )
                        nc.tensor.matmul(
                            ps_box[0][:],
                            Utn_sb[k][nt // 4][:, (nt % 4) * 128 : (nt % 4 + 1) * 128],
                            wout_sb[k][:],
                            start=(k == 0),
                            stop=(k == KT - 1),
                        )
                        if k == KT - 1:
                            osb = opool.tile([128, D], f32, tag="osb", name=f"osb{nt}")
                            psum_copy(osb[:], ps_box[0][:])
                            nc.sync.dma_start(out[nt * 128 : (nt + 1) * 128, :], osb[:])
                    return go

                return [mm(k) for k in range(KT)]

            def run_group(mms):
                for fn in mms:
                    fn()

            # Minimal pre-attention work: only what block (ib0, t0) needs up
            # front. Everything else is interleaved at scheduled (block, jt)
            # slots, ~2 matmuls per slot, so it rides in the PE's idle time
            # while ScalarE streams exps.
            run_group(qk_proj_mms(0, 0, 0))  # QT[t0] i-cols 0:512
            run_group(qk_proj_mms(0, 0, 1))  # KT[t0] j-cols 0:512
            run_group(qk_proj_mms(0, 1, 1))  # KT[t0] j-cols 512:1024
            run_group(v_proj_mms(0))
            run_group(v_proj_mms(1))
            run_group(v_proj_mms(2))

            blocks = [(ib, t) for ib in range(NIB) for t in range(KT)]
            # tasks[(bi, jt)] = list of closures (individual matmuls/copies)
            tasks = {}

            def sched(bi, jt, mms, per_slot=2):
                # spread a group's matmuls over consecutive jt slots,
                # per_slot per slot starting at (bi, jt).  NOTE: a group
                # consumed by the next block's pre-issued QK must fully land
                # by slot (bi, 6) - slot (bi, 7)'s tasks are emitted after
                # the pre_qk for block bi+1.
                for i, fn in enumerate(mms):
                    slot = jt + i // per_slot
                    b2, j2 = bi + slot // NJT, slot % NJT
                    tasks.setdefault((b2, j2), []).append(fn)

            # remaining V projections in block 0 (PE has slack there while
            # the exp chain ramps): v_proj(nt) complete before AV(jt=nt)
            for nt in range(3, NJT):
                sched(0, nt - 3, v_proj_mms(nt), per_slot=4)
            # Q^T/K^T ib0 for pair tn, finishing before block tn's first QK
            # (pre-issued at (tn-1, jt=7))
            sched(0, 5, qk_proj_mms(1, 0, 0), per_slot=4)
            sched(0, 6, qk_proj_mms(1, 0, 1), per_slot=4)
            for bi, tn in ((1, 2), (2, 3)):
                sched(bi, 3, qk_proj_mms(tn, 0, 0))
                sched(bi, 4, qk_proj_mms(tn, 0, 1))
            # K^T j-cols 512:1024 of pair tn, needed from block tn's jt=4
            for bi, tn in ((0, 1), (1, 2), (2, 3)):
                sched(bi + 1, 1, qk_proj_mms(tn, 1, 1))
            for bi in range(4):
                # QT i-cols 512:1024 of pair bi, needed from block 4+bi,
                # whose first QK pre-issues at (3+bi, jt=7)
                sched(bi + 1, 5, qk_proj_mms(bi, 1, 0))
            # wout loads on the scalar queue after the first eb tiles (slot
            # (0,5) -> trigger lands behind eb0-3), well before block 5
            def load_wout():
                for k in range(KT):
                    nc.scalar.dma_start(
                        wout_sb[k][:], wout[k * 128 : (k + 1) * 128, :]
                    )

            tasks.setdefault((0, 5), []).insert(0, load_wout)
            # out projections for the ib=0 half: Utn[*][0] ready after block
            # 3's norms (flushed at block 4, jt=1)
            for nt in range(4):
                sched(5 + nt // 2, 2 + 3 * (nt % 2), out_proj_mms(nt))
            # ib=1 half, k-tiles 0..2: Utn[0..2][1] are ready once block 6's
            # norms flush at (7,1) - pre-accumulate nt=4,5 during block 7 so
            # only their k=3 matmul (plus nt=6,7) remains after the final
            # norm.  mm_ps has exactly 2 free slots alongside block 7's ups.
            tail_pre = {nt: out_proj_mms(nt) for nt in range(4, NJT)}
            for i, nt in enumerate((4, 5)):
                sched(7, 2 + 2 * i, tail_pre[nt][:KT - 1], per_slot=2)

            # ---- Phase 2: attention (transposed), even/odd heads paired ----
            # The two heads of pair t sit at partitions 0:64 / 64:128 of
            # QT_sb[t]/KT_sb[t]. One exp / one bias-multiply covers both.
            pending_norms = []

            def flush_norms():
                while pending_norms:
                    pending_norms.pop(0)()

            def make_qk(t, ib):
                def qk(jt):
                    st = st_ps.tile(
                        [128, 2 * IB], f32, bufs=2, tag="st", name=f"st{t}{ib}{jt}"
                    )
                    nc.tensor.matmul(
                        st[:, 0:IB],
                        KT_sb[t][0:64, jt * 128 : (jt + 1) * 128],
                        QT_sb[t][0:64, ib * IB : (ib + 1) * IB],
                        start=True,
                        stop=True,
                    )
                    nc.tensor.matmul(
                        st[:, IB : 2 * IB],
                        KT_sb[t][64:128, jt * 128 : (jt + 1) * 128],
                        QT_sb[t][64:128, ib * IB : (ib + 1) * IB],
                        start=True,
                        stop=True,
                    )
                    return st
                return qk

            pre_qk = None
            for bi, (ib, t) in enumerate(blocks):
                he, ho = 2 * t, 2 * t + 1
                ups_e = mm_ps.tile([128, IB], f32, tag="ps", name=f"upse{t}{ib}")
                ups_o = mm_ps.tile([128, IB], f32, tag="ps", name=f"upso{t}{ib}")
                qk = make_qk(t, ib)

                def make_av(jt, et, ups_e=ups_e, ups_o=ups_o, he=he, ho=ho):
                    def go():
                        nc.tensor.matmul(
                            ups_e[:],
                            V1_sb[jt][:, he * 128 : (he + 1) * 128],
                            et[:, 0:IB],
                            start=(jt == 0),
                            stop=(jt == NJT - 1),
                        )
                        nc.tensor.matmul(
                            ups_o[:],
                            V1_sb[jt][:, ho * 128 : (ho + 1) * 128],
                            et[:, IB : 2 * IB],
                            start=(jt == 0),
                            stop=(jt == NJT - 1),
                        )
                    return go

                # software pipeline: QK(jt+1) issues on PE before AV(jt), and
                # AV(jt) is deferred a full slot (emitted at jt+1, behind the
                # filler tasks) - the PE queue is in-order, so this gives the
                # QK->exp->mul chain ~2 steps of latency budget before an
                # unready et can block the PE.
                sts = [pre_qk] if pre_qk is not None else [qk(0)]
                pre_qk = None
                pend_av = None
                for jt in range(NJT):
                    if jt + 1 < NJT:
                        sts.append(qk(jt + 1))
                    elif bi + 1 < len(blocks):
                        nib, nt_ = blocks[bi + 1]
                        pre_qk = make_qk(nt_, nib)(0)
                    st = sts[jt]
                    eb = load_eb(t, ib, jt)
                    et0 = stream.tile([128, 2 * IB], f16, tag="et0", bufs=5)
                    nc.scalar.activation(et0[:], st[:], Exp)
                    et = stream.tile([128, 2 * IB], f16, tag="et", bufs=5)
                    nc.vector.tensor_mul(et[:], et0[:], eb[:])
                    if jt in (1, 2) and pending_norms:
                        # one half of the previous block's norms per slot,
                        # deferred + split so the DVE never sees a >1.5us
                        # burst between this block's et-multiplies
                        pending_norms.pop(0)()
                    for fn in tasks.get((bi, jt), ()):
                        fn()
                    if pend_av is not None:
                        pend_av()
                    pend_av = make_av(jt, et)
                pend_av()

                def make_norm(po, ups, t=t, ib=ib):
                    def go():
                        rb = stream.tile(
                            [64, IB], f32, tag="rb", name=f"rb{t}{ib}{po}"
                        )
                        nc.vector.reciprocal_approx_fast(rb[:, :], ups[0:64, :])
                        nc.vector.tensor_mul(
                            Utn_sb[t][ib][po : po + 64, :],
                            ups[64:128, :],
                            rb[:, :],
                        )
                    return go

                pending_norms.append(make_norm(0, ups_e))
                pending_norms.append(make_norm(64, ups_o))
            flush_norms()

            # ---- Phase 3: remaining output projections (ib=1 half) ----
            for nt in (4, 5):
                run_group(tail_pre[nt][KT - 1 :])
            for nt in (6, 7):
                run_group(tail_pre[nt])

    return nc


def _get_graph():
    if "nc" not in _CACHE:
        nc = _build_graph()
        nc.compile()
        _CACHE["nc"] = nc
    return _CACHE["nc"]


def _prep_inputs(x, pos_bias, w_qkv, w_out):
    x = np.asarray(x, dtype=np.float32)
    pos_bias = np.asarray(pos_bias, dtype=np.float32)
    w_qkv = np.asarray(w_qkv, dtype=np.float32)
    w_out = np.asarray(w_out, dtype=np.float32)

    wqkv_mod = w_qkv.copy()
    wqkv_mod[:, :D] *= SCALE
    wout16 = w_out.astype(np.float16)
    wqkv16 = wqkv_mod.astype(np.float16)
    # prepacked exp(bias^T) tiles: ebt[t, ib, jt] = [128 j, he-i | ho-i]
    ebt = np.exp(pos_bias.transpose(0, 2, 1)).astype(np.float16)  # [h, j, i]
    ebt4 = ebt.reshape(KT, 2, NJT, 128, NIB, IB)  # [t, par, jt, p, ib, i]
    ebt_tiles = np.ascontiguousarray(
        ebt4.transpose(0, 4, 2, 3, 1, 5).reshape(KT, NIB, NJT, 128, 2 * IB)
    )

    in_maps = []
    for b in range(NCORES):
        in_maps.append(
            {
                "xT": np.ascontiguousarray(x[b].T.astype(np.float16)),
                "wqkv": wqkv16,
                "wout": wout16,
                "ebt": ebt_tiles,
            }
        )
    return in_maps


def _run(x, pos_bias, w_qkv, w_out, trace=False):
    from concourse.bass_utils import run_bass_kernel_spmd

    nc = _get_graph()
    in_maps = _prep_inputs(x, pos_bias, w_qkv, w_out)
    res = run_bass_kernel_spmd(
        nc, in_maps, core_ids=list(range(NCORES)), trace=trace
    )
    outs = np.stack([np.asarray(res.results[b]["out"]) for b in range(NCORES)])
    return outs.astype(np.float32), res


def kernel(x, pos_bias, w_qkv, w_out):
    outs, _ = _run(x, pos_bias, w_qkv, w_out, trace=False)
    return outs


# revision 29
# speedup vs baseline: 1.0174x; 1.0128x over previous
"""Distributed Trainium2 kernel for batched multi-head self-attention with
positional bias.

Reference computation (per batch element b):
    qkv = x[b] @ w_qkv ; split into q,k,v ; heads of 64
    sim = (q * 64**-0.5) @ k^T + pos_bias          # [h, n, n]
    attn = softmax(sim, axis=-1)
    out[b] = (attn @ v).reshape(n, hidden) @ w_out

Sharding: pure data-parallel - core i computes batch element i (B == 8 ==
n_cores), no collectives.

Device algorithm (per core), designed to avoid all on-chip transposes:
  - host supplies xT = x[b].T, so projections produce Q^T,K^T ([d, n]) and V
    ([n, d]) directly with natural-layout matmuls.
  - attention is computed transposed: St[j,i] = sum_d K^T[d,j] Q^T[d,i];
    softmax over j is handled via exp (ScalarE) * exp(bias^T) (host
    precomputed, fp16, prepacked per-tile) and a ones-block in the AV
    matmul's stationary operand, which makes PSUM rows 0:64 the softmax
    denominators.
  - U''[64:128] * 1/U''[0:64] gives the normalized per-head context, already
    in the [hidden, n] layout the output projection needs as lhsT.

Scheduling (v2): fine-grained input DMAs split across the sync and scalar
HW-DGE queues so the first projections and first bias tiles land ~5us
earlier; projection matmuls are spread ~2 per attention step as PE gap
filler (the exp->mul chain latency otherwise stalls the AV matmuls); PSUM
drains alternate ScalarE/VectorE.
"""

import numpy as np

B, N, D = 8, 1024, 512
H, DH = 8, 64
SCALE = DH**-0.5
NCORES = 8
KT = D // 128  # 4 k-tiles over model dim / hidden dim
NJT = N // 128  # 8 j-tiles
IB = 512
NIB = N // IB  # 2 i-blocks
NWARM = 12

_CACHE = {}


def _build_graph(sim=False):
    import concourse.bass as bass
    import concourse.mybir as mybir
    from concourse import tile

    f32 = mybir.dt.float32
    f16 = mybir.dt.float16
    Exp = mybir.ActivationFunctionType.Exp

    import concourse.bacc as bacc

    # target_bir_lowering=False: bass/bacc lower to per-engine streams with
    # standalone waits itself; walrus's sync structs hold few waits and
    # reject Tile-generated multi-wait instructions otherwise.
    nc = bacc.Bacc(None, target_bir_lowering=False, debug=False)
    xT = nc.declare_dram_parameter("xT", [D, N], f16, isOutput=False)
    wqkv = nc.declare_dram_parameter("wqkv", [D, 3 * D], f16, isOutput=False)
    wout = nc.declare_dram_parameter("wout", [D, D], f16, isOutput=False)
    # host-prepacked exp(bias^T) tiles: ebt[t, ib, jt] = [128 j, he-i | ho-i]
    ebt = nc.declare_dram_parameter(
        "ebt", [KT, NIB, NJT, 128, 2 * IB], f16, isOutput=False
    )
    out = nc.declare_dram_parameter("out", [N, D], f32, isOutput=True)

    with tile.TileContext(nc) as tc:
        with (
            tc.tile_pool(name="const", bufs=1) as cpool,
            tc.tile_pool(name="mm_ps", bufs=4, space="PSUM") as mm_ps,
            tc.tile_pool(name="st_ps", bufs=2, space="PSUM") as st_ps,
            tc.tile_pool(name="stream", bufs=4) as stream,
            tc.tile_pool(name="osb", bufs=2) as opool,
        ):
            # ---- Phase 0: resident allocation + fine-grained loads ----
            # sync queue: xT k-tiles, then w q-cols, then w k-cols, then the
            # steady eb stream.  scalar queue (idle engine at start): w
            # v-cols, the first 4 eb tiles, and wout - all off the critical
            # path of the first Q/K projections.
            w_sb = [
                cpool.tile([128, 3 * D], f16, tag=f"w{k}", name=f"w{k}")
                for k in range(KT)
            ]
            xT_sb = [
                cpool.tile([128, N], f16, tag=f"xt{k}", name=f"xt{k}")
                for k in range(KT)
            ]
            wout_sb = [
                cpool.tile([128, D], f16, tag=f"wo{k}", name=f"wo{k}")
                for k in range(KT)
            ]
            # xT is split across both queues so the 1.25MB critical startup
            # set (xT + the t=0 column slices of w_q / w_k, which gate the
            # first Q/K projections) finishes at the HBM-bus floor; the
            # t=1..3 w slices follow and are needed ~10us later
            for k in (0, 1):
                nc.sync.dma_start(xT_sb[k][:], xT[k * 128 : (k + 1) * 128, :])
            for k in (2, 3):
                nc.scalar.dma_start(xT_sb[k][:], xT[k * 128 : (k + 1) * 128, :])
            for c0, c1 in ((0, 128), (D, D + 128), (128, D), (D + 128, 2 * D)):
                for k in range(KT):
                    nc.sync.dma_start(
                        w_sb[k][:, c0:c1], wqkv[k * 128 : (k + 1) * 128, c0:c1]
                    )
            # v-cols first on the scalar queue: v_proj(0..2) run right after
            # the three upfront q/k projection groups
            for k in range(KT):
                nc.scalar.dma_start(
                    w_sb[k][:, 2 * D : 3 * D],
                    wqkv[k * 128 : (k + 1) * 128, 2 * D : 3 * D],
                )

            # V1: per jt a [128, H*128] tensor holding, per head, the AV
            # stationary operand [ones | v_h] (ones via memset, v written by
            # the V projection).
            V1_sb = []
            for jt in range(NJT):
                v1 = cpool.tile([128, H * 128], f16, tag=f"v1_{jt}", name=f"v1_{jt}")
                nc.gpsimd.memset(v1[:], 1.0)
                V1_sb.append(v1)

            QT_sb = [cpool.tile([128, N], f16, tag=f"qt{t}", name=f"qt{t}") for t in range(KT)]
            KT_sb = [cpool.tile([128, N], f16, tag=f"kt{t}", name=f"kt{t}") for t in range(KT)]
            Utn_sb = [
                [
                    cpool.tile([128, IB], f16, tag=f"ut{t}_{ib}", name=f"ut{t}_{ib}")
                    for ib in range(NIB)
                ]
                for t in range(KT)
            ]

            # eb DMA launcher: first block's tiles ride the scalar queue
            # (issued during startup), everything else the sync queue.
            eb_count = [0]

            def load_eb(t, ib, jt):
                eb = stream.tile([128, 2 * IB], f16, tag="eb", bufs=6)
                eng = nc.scalar if eb_count[0] < 4 else nc.sync
                eb_count[0] += 1
                eng.dma_start(eb[:], ebt[t, ib, jt, :, :])
                return eb

            # ---- Phase 0b: PE warm-up + ACT exp-table preload during the
            # input-DMA window. Dummy matmuls keep the PE HAM busy while
            # xT/w DMAs land, so real matmuls start at 2.4 GHz.
            dumA = cpool.tile([128, 128], f16, tag="dumA", name="dumA")
            dumB = cpool.tile([128, 512], f16, tag="dumB", name="dumB")
            dumE = cpool.tile([128, 64], f16, tag="dumE", name="dumE")
            nc.gpsimd.memset(dumA[:], 0.0)
            nc.gpsimd.memset(dumB[:], 0.0)
            # exp-table preload so the first real exp doesn't pay the
            # ~1.3us ACT_TABLE_LOAD
            nc.scalar.activation(dumE[:], dumB[:, 0:64], Exp)
            for i in range(NWARM):
                wps = mm_ps.tile([128, IB], f32, tag="ps", name=f"warm{i}")
                nc.tensor.matmul(wps[:], dumA[:], dumB[:], start=True, stop=True)

            copy_count = [0]

            def psum_copy(dst, src):
                # PSUM->SBUF drains alternate ScalarE/VectorE so neither
                # pacing engine absorbs them all.
                copy_count[0] += 1
                if copy_count[0] % 2:
                    nc.scalar.copy(dst, src)
                else:
                    nc.vector.tensor_copy(dst, src)

            def qk_proj_mms(t, ib, which, copy_eng=None):
                # one Q^T (which=0) or K^T (which=1) projection group as a
                # list of single-matmul closures + final copy closure, so the
                # scheduler can spread them across attention steps.
                dst = QT_sb if which == 0 else KT_sb
                col0 = 0 if which == 0 else D
                ps_box = []

                def mm(k):
                    def go():
                        if k == 0:
                            ps_box.append(
                                mm_ps.tile([128, IB], f32, tag="ps", name=f"qkp{t}{ib}{which}")
                            )
                        nc.tensor.matmul(
                            ps_box[0][:],
                            w_sb[k][:, col0 + t * 128 : col0 + (t + 1) * 128],
                            xT_sb[k][:, ib * IB : (ib + 1) * IB],
                            start=(k == 0),
                            stop=(k == KT - 1),
                        )
                        if k == KT - 1:
                            d = dst[t][:, ib * IB : (ib + 1) * IB]
                            if copy_eng == "scalar":
                                nc.scalar.copy(d, ps_box[0][:])
                            elif copy_eng == "vector":
                                nc.vector.tensor_copy(d, ps_box[0][:])
                            else:
                                psum_copy(d, ps_box[0][:])
                    return go

                return [mm(k) for k in range(KT)]

            def v_proj_mms(nt):
                ps_box = []

                def mm(k):
                    def go():
                        if k == 0:
                            ps_box.append(
                                mm_ps.tile([128, D], f32, tag="ps", name=f"vps{nt}")
                            )
                        nc.tensor.matmul(
                            ps_box[0][:],
                            xT_sb[k][:, nt * 128 : (nt + 1) * 128],
                            w_sb[k][:, 2 * D : 3 * D],
                            start=(k == 0),
                            stop=(k == KT - 1),
                        )
                        if k == KT - 1:
                            nc.vector.tensor_copy(
                                V1_sb[nt].rearrange("p (h c) -> p h c", h=H)[:, :, DH : 2 * DH],
                                ps_box[0].rearrange("p (h c) -> p h c", h=H)[:, :, :],
                            )
                    return go

                return [mm(k) for k in range(KT)]

            def out_proj_mms(nt, pool=None, tag="ps"):
                ps_box = []
                pool_ = pool if pool is not None else mm_ps

                def mm(k):
                    def go():
                        if k == 0:
                            ps_box.append(
                                pool_.tile([128, D], f32, tag=tag, name=f"ops{nt}")
                            )
                        nc.tensor.matmul(
                            ps_box[0][:],
                            Utn_sb[k][nt // 4][:, (nt % 4) * 128 : (nt % 4 + 1) * 128],
                            wout_sb[k][:],
                            start=(k == 0),
                            stop=(k == KT - 1),
                        )
                        if k == KT - 1:
                            osb = opool.tile([128, D], f32, tag="osb", name=f"osb{nt}")
                            psum_copy(osb[:], ps_box[0][:])
                            nc.sync.dma_start(out[nt * 128 : (nt + 1) * 128, :], osb[:])
                    return go

                return [mm(k) for k in range(KT)]

            def run_group(mms):
                for fn in mms:
                    fn()

            # Minimal pre-attention work: only what block (ib0, t0) needs up
            # front. Everything else is interleaved at scheduled (block, jt)
            # slots, ~2 matmuls per slot, so it rides in the PE's idle time
            # while ScalarE streams exps.
            run_group(qk_proj_mms(0, 0, 0))  # QT[t0] i-cols 0:512
            run_group(qk_proj_mms(0, 0, 1))  # KT[t0] j-cols 0:512
            run_group(qk_proj_mms(0, 1, 1))  # KT[t0] j-cols 512:1024
            run_group(v_proj_mms(0))
            run_group(v_proj_mms(1))
            run_group(v_proj_mms(2))

            blocks = [(ib, t) for ib in range(NIB) for t in range(KT)]
            # tasks[(bi, jt)] = list of closures (individual matmuls/copies)
            tasks = {}

            def sched(bi, jt, mms, per_slot=2):
                # spread a group's matmuls over consecutive jt slots,
                # per_slot per slot starting at (bi, jt).  NOTE: a group
                # consumed by the next block's pre-issued QK must fully land
                # by slot (bi, 6) - slot (bi, 7)'s tasks are emitted after
                # the pre_qk for block bi+1.
                for i, fn in enumerate(mms):
                    slot = jt + i // per_slot
                    b2, j2 = bi + slot // NJT, slot % NJT
                    tasks.setdefault((b2, j2), []).append(fn)

            # remaining V projections in block 0 (PE has slack there while
            # the exp chain ramps): v_proj(nt) complete before AV(jt=nt)
            for nt in range(3, NJT):
                sched(0, nt - 3, v_proj_mms(nt), per_slot=4)
            # Q^T/K^T ib0 for pair tn, finishing before block tn's first QK
            # (pre-issued at (tn-1, jt=7))
            sched(0, 5, qk_proj_mms(1, 0, 0), per_slot=4)
            sched(0, 6, qk_proj_mms(1, 0, 1), per_slot=4)
            for bi, tn in ((1, 2), (2, 3)):
                sched(bi, 3, qk_proj_mms(tn, 0, 0))
                sched(bi, 4, qk_proj_mms(tn, 0, 1))
            # K^T j-cols 512:1024 of pair tn, needed from block tn's jt=4.
            # These groups land in slots 1-2 where the DVE already runs the
            # previous block's norm pairs - pin their PSUM-drain copy to
            # ScalarE so it doesn't head-of-line block the et-multiplies.
            for tn in (1, 2, 3):
                sched(tn, 1, qk_proj_mms(tn, 1, 1, copy_eng="scalar"))
            for bi in range(4):
                # QT i-cols 512:1024 of pair bi, needed from block 4+bi,
                # whose first QK pre-issues at (3+bi, jt=7)
                sched(bi + 1, 5, qk_proj_mms(bi, 1, 0))
            # wout loads on the scalar queue after the first eb tiles (slot
            # (0,5) -> trigger lands behind eb0-3), well before block 5
            def load_wout():
                for k in range(KT):
                    nc.scalar.dma_start(
                        wout_sb[k][:], wout[k * 128 : (k + 1) * 128, :]
                    )

            tasks.setdefault((0, 5), []).insert(0, load_wout)
            # out projections for the ib=0 half: Utn[*][0] ready after block
            # 3's norms (flushed at block 4, jt=1)
            for nt in range(4):
                sched(5 + nt // 2, 2 + 3 * (nt % 2), out_proj_mms(nt))
            # ib=1 half, k-tiles 0..2: Utn[0..2][1] are ready once block 6's
            # norms flush at (7,1) - pre-accumulate nt=4,5 during block 7 so
            # only their k=3 matmul (plus nt=6,7) remains after the final
            # norm.  mm_ps has exactly 2 free slots alongside block 7's ups.
            tail_pre = {nt: out_proj_mms(nt) for nt in (4, 5)}
            for nt in (6, 7):
                # nt=6,7 accumulate in st_ps slots, which free up as block
                # 7's last exps drain - their k=0..2 matmuls fill the PE's
                # tail window while the final norms run
                tail_pre[nt] = out_proj_mms(nt, pool=st_ps, tag="st")
            for i, nt in enumerate((4, 5)):
                sched(7, 2 + 2 * i, tail_pre[nt][:KT - 1], per_slot=2)

            # ---- Phase 2: attention (transposed), even/odd heads paired ----
            # The two heads of pair t sit at partitions 0:64 / 64:128 of
            # QT_sb[t]/KT_sb[t]. One exp / one bias-multiply covers both.
            pending_norms = []

            def flush_norms():
                while pending_norms:
                    pending_norms.pop(0)()

            def make_qk(t, ib):
                def qk(jt):
                    st = st_ps.tile(
                        [128, 2 * IB], f32, bufs=2, tag="st", name=f"st{t}{ib}{jt}"
                    )
                    nc.tensor.matmul(
                        st[:, 0:IB],
                        KT_sb[t][0:64, jt * 128 : (jt + 1) * 128],
                        QT_sb[t][0:64, ib * IB : (ib + 1) * IB],
                        start=True,
                        stop=True,
                    )
                    nc.tensor.matmul(
                        st[:, IB : 2 * IB],
                        KT_sb[t][64:128, jt * 128 : (jt + 1) * 128],
                        QT_sb[t][64:128, ib * IB : (ib + 1) * IB],
                        start=True,
                        stop=True,
                    )
                    return st
                return qk

            pre_qk = None
            for bi, (ib, t) in enumerate(blocks):
                he, ho = 2 * t, 2 * t + 1
                ups_e = mm_ps.tile([128, IB], f32, tag="ps", name=f"upse{t}{ib}")
                ups_o = mm_ps.tile([128, IB], f32, tag="ps", name=f"upso{t}{ib}")
                qk = make_qk(t, ib)

                def make_av(jt, et, ups_e=ups_e, ups_o=ups_o, he=he, ho=ho):
                    def go():
                        nc.tensor.matmul(
                            ups_e[:],
                            V1_sb[jt][:, he * 128 : (he + 1) * 128],
                            et[:, 0:IB],
                            start=(jt == 0),
                            stop=(jt == NJT - 1),
                        )
                        nc.tensor.matmul(
                            ups_o[:],
                            V1_sb[jt][:, ho * 128 : (ho + 1) * 128],
                            et[:, IB : 2 * IB],
                            start=(jt == 0),
                            stop=(jt == NJT - 1),
                        )
                    return go

                # software pipeline: QK(jt+1) issues on PE before AV(jt), and
                # AV(jt) is deferred a full slot (emitted at jt+1, behind the
                # filler tasks) - the PE queue is in-order, so this gives the
                # QK->exp->mul chain ~2 steps of latency budget before an
                # unready et can block the PE.
                sts = [pre_qk] if pre_qk is not None else [qk(0)]
                pre_qk = None
                pend_av = None
                for jt in range(NJT):
                    if jt + 1 < NJT:
                        sts.append(qk(jt + 1))
                    elif bi + 1 < len(blocks):
                        nib, nt_ = blocks[bi + 1]
                        pre_qk = make_qk(nt_, nib)(0)
                    st = sts[jt]
                    eb = load_eb(t, ib, jt)
                    et0 = stream.tile([128, 2 * IB], f16, tag="et0", bufs=5)
                    nc.scalar.activation(et0[:], st[:], Exp)
                    et = stream.tile([128, 2 * IB], f16, tag="et", bufs=5)
                    nc.vector.tensor_mul(et[:], et0[:], eb[:])
                    if jt in (1, 2) and pending_norms:
                        # one half of the previous block's norms per slot,
                        # deferred + split so the DVE never sees a >1.5us
                        # burst between this block's et-multiplies
                        pending_norms.pop(0)()
                    for fn in tasks.get((bi, jt), ()):
                        fn()
                    if pend_av is not None:
                        pend_av()
                    pend_av = make_av(jt, et)
                if bi + 1 < len(blocks):
                    pend_av()
                else:
                    # last block: slot the nt=6,7 out-projection partials
                    # around the final AV so the PE tail window stays full
                    for fn in tail_pre[6][: KT - 1]:
                        fn()
                    pend_av()
                    for fn in tail_pre[7][: KT - 1]:
                        fn()

                def make_norm(po, ups, t=t, ib=ib):
                    def go():
                        rb = stream.tile(
                            [64, IB], f32, tag="rb", name=f"rb{t}{ib}{po}"
                        )
                        nc.vector.reciprocal_approx_fast(rb[:, :], ups[0:64, :])
                        nc.vector.tensor_mul(
                            Utn_sb[t][ib][po : po + 64, :],
                            ups[64:128, :],
                            rb[:, :],
                        )
                    return go

                pending_norms.append(make_norm(0, ups_e))
                pending_norms.append(make_norm(64, ups_o))
            flush_norms()

            # ---- Phase 3: final k-tile of each remaining output projection ----
            for nt in (4, 5, 6, 7):
                run_group(tail_pre[nt][KT - 1 :])

    return nc


def _get_graph():
    if "nc" not in _CACHE:
        nc = _build_graph()
        nc.compile()
        _CACHE["nc"] = nc
    return _CACHE["nc"]


def _prep_inputs(x, pos_bias, w_qkv, w_out):
    x = np.asarray(x, dtype=np.float32)
    pos_bias = np.asarray(pos_bias, dtype=np.float32)
    w_qkv = np.asarray(w_qkv, dtype=np.float32)
    w_out = np.asarray(w_out, dtype=np.float32)

    wqkv_mod = w_qkv.copy()
    wqkv_mod[:, :D] *= SCALE
    wout16 = w_out.astype(np.float16)
    wqkv16 = wqkv_mod.astype(np.float16)
    # prepacked exp(bias^T) tiles: ebt[t, ib, jt] = [128 j, he-i | ho-i]
    ebt = np.exp(pos_bias.transpose(0, 2, 1)).astype(np.float16)  # [h, j, i]
    ebt4 = ebt.reshape(KT, 2, NJT, 128, NIB, IB)  # [t, par, jt, p, ib, i]
    ebt_tiles = np.ascontiguousarray(
        ebt4.transpose(0, 4, 2, 3, 1, 5).reshape(KT, NIB, NJT, 128, 2 * IB)
    )

    in_maps = []
    for b in range(NCORES):
        in_maps.append(
            {
                "xT": np.ascontiguousarray(x[b].T.astype(np.float16)),
                "wqkv": wqkv16,
                "wout": wout16,
                "ebt": ebt_tiles,
            }
        )
    return in_maps


def _run(x, pos_bias, w_qkv, w_out, trace=False):
    from concourse.bass_utils import run_bass_kernel_spmd

    nc = _get_graph()
    in_maps = _prep_inputs(x, pos_bias, w_qkv, w_out)
    res = run_bass_kernel_spmd(
        nc, in_maps, core_ids=list(range(NCORES)), trace=trace
    )
    outs = np.stack([np.asarray(res.results[b]["out"]) for b in range(NCORES)])
    return outs.astype(np.float32), res


def kernel(x, pos_bias, w_qkv, w_out):
    outs, _ = _run(x, pos_bias, w_qkv, w_out, trace=False)
    return outs


# revision 30
# speedup vs baseline: 1.0293x; 1.0116x over previous
"""Distributed Trainium2 kernel for batched multi-head self-attention with
positional bias.

Reference computation (per batch element b):
    qkv = x[b] @ w_qkv ; split into q,k,v ; heads of 64
    sim = (q * 64**-0.5) @ k^T + pos_bias          # [h, n, n]
    attn = softmax(sim, axis=-1)
    out[b] = (attn @ v).reshape(n, hidden) @ w_out

Sharding: pure data-parallel - core i computes batch element i (B == 8 ==
n_cores), no collectives.

Device algorithm (per core), designed to avoid all on-chip transposes:
  - host supplies xT = x[b].T, so projections produce Q^T,K^T ([d, n]) and V
    ([n, d]) directly with natural-layout matmuls.
  - attention is computed transposed: St[j,i] = sum_d K^T[d,j] Q^T[d,i];
    softmax over j is handled via exp (ScalarE) * exp(bias^T) (host
    precomputed, fp16, prepacked per-tile) and a ones-block in the AV
    matmul's stationary operand, which makes PSUM rows 0:64 the softmax
    denominators.
  - U''[64:128] * 1/U''[0:64] gives the normalized per-head context, already
    in the [hidden, n] layout the output projection needs as lhsT.

Scheduling (v2): fine-grained input DMAs split across the sync and scalar
HW-DGE queues so the first projections and first bias tiles land ~5us
earlier; projection matmuls are spread ~2 per attention step as PE gap
filler (the exp->mul chain latency otherwise stalls the AV matmuls); PSUM
drains alternate ScalarE/VectorE.
"""

import numpy as np

B, N, D = 8, 1024, 512
H, DH = 8, 64
SCALE = DH**-0.5
NCORES = 8
KT = D // 128  # 4 k-tiles over model dim / hidden dim
NJT = N // 128  # 8 j-tiles
IB = 512
NIB = N // IB  # 2 i-blocks
NWARM = 12

_CACHE = {}


def _build_graph(sim=False):
    import concourse.bass as bass
    import concourse.mybir as mybir
    from concourse import tile

    f32 = mybir.dt.float32
    f16 = mybir.dt.float16
    Exp = mybir.ActivationFunctionType.Exp

    import concourse.bacc as bacc

    # target_bir_lowering=False: bass/bacc lower to per-engine streams with
    # standalone waits itself; walrus's sync structs hold few waits and
    # reject Tile-generated multi-wait instructions otherwise.
    nc = bacc.Bacc(None, target_bir_lowering=False, debug=False)
    xT = nc.declare_dram_parameter("xT", [D, N], f16, isOutput=False)
    wqkv = nc.declare_dram_parameter("wqkv", [D, 3 * D], f16, isOutput=False)
    wout = nc.declare_dram_parameter("wout", [D, D], f16, isOutput=False)
    # host-prepacked exp(bias^T) tiles: ebt[t, ib, jt] = [128 j, he-i | ho-i]
    ebt = nc.declare_dram_parameter(
        "ebt", [KT, NIB, NJT, 128, 2 * IB], f16, isOutput=False
    )
    out = nc.declare_dram_parameter("out", [N, D], f32, isOutput=True)

    with tile.TileContext(nc) as tc:
        with (
            tc.tile_pool(name="const", bufs=1) as cpool,
            tc.tile_pool(name="mm_ps", bufs=4, space="PSUM") as mm_ps,
            tc.tile_pool(name="st_ps", bufs=2, space="PSUM") as st_ps,
            tc.tile_pool(name="stream", bufs=4) as stream,
            tc.tile_pool(name="osb", bufs=2) as opool,
        ):
            # ---- Phase 0: resident allocation + fine-grained loads ----
            # sync queue: xT k-tiles, then w q-cols, then w k-cols, then the
            # steady eb stream.  scalar queue (idle engine at start): w
            # v-cols, the first 4 eb tiles, and wout - all off the critical
            # path of the first Q/K projections.
            w_sb = [
                cpool.tile([128, 3 * D], f16, tag=f"w{k}", name=f"w{k}")
                for k in range(KT)
            ]
            xT_sb = [
                cpool.tile([128, N], f16, tag=f"xt{k}", name=f"xt{k}")
                for k in range(KT)
            ]
            wout_sb = [
                cpool.tile([128, D], f16, tag=f"wo{k}", name=f"wo{k}")
                for k in range(KT)
            ]
            # xT is split across both queues so the 1.25MB critical startup
            # set (xT + the t=0 column slices of w_q / w_k, which gate the
            # first Q/K projections) finishes at the HBM-bus floor; the
            # t=1..3 w slices follow and are needed ~10us later
            for k in (0, 1):
                nc.sync.dma_start(xT_sb[k][:], xT[k * 128 : (k + 1) * 128, :])
            for k in (2, 3):
                nc.scalar.dma_start(xT_sb[k][:], xT[k * 128 : (k + 1) * 128, :])
            for c0, c1 in ((0, 128), (D, D + 128), (128, D), (D + 128, 2 * D)):
                for k in range(KT):
                    nc.sync.dma_start(
                        w_sb[k][:, c0:c1], wqkv[k * 128 : (k + 1) * 128, c0:c1]
                    )
            # v-cols first on the scalar queue: v_proj(0..2) run right after
            # the three upfront q/k projection groups
            for k in range(KT):
                nc.scalar.dma_start(
                    w_sb[k][:, 2 * D : 3 * D],
                    wqkv[k * 128 : (k + 1) * 128, 2 * D : 3 * D],
                )

            # V1: per jt a [128, H*128] tensor holding, per head, the AV
            # stationary operand [ones | v_h] (ones via memset, v written by
            # the V projection).
            V1_sb = []
            for jt in range(NJT):
                v1 = cpool.tile([128, H * 128], f16, tag=f"v1_{jt}", name=f"v1_{jt}")
                nc.gpsimd.memset(v1[:], 1.0)
                V1_sb.append(v1)

            QT_sb = [cpool.tile([128, N], f16, tag=f"qt{t}", name=f"qt{t}") for t in range(KT)]
            KT_sb = [cpool.tile([128, N], f16, tag=f"kt{t}", name=f"kt{t}") for t in range(KT)]
            Utn_sb = [
                [
                    cpool.tile([128, IB], f16, tag=f"ut{t}_{ib}", name=f"ut{t}_{ib}")
                    for ib in range(NIB)
                ]
                for t in range(KT)
            ]

            # eb DMA launcher: first block's tiles ride the scalar queue
            # (issued during startup), everything else the sync queue.
            eb_count = [0]

            def load_eb(t, ib, jt):
                eb = stream.tile([128, 2 * IB], f16, tag="eb", bufs=6)
                eng = nc.scalar if eb_count[0] < 4 else nc.sync
                eb_count[0] += 1
                eng.dma_start(eb[:], ebt[t, ib, jt, :, :])
                return eb

            # ---- Phase 0b: PE warm-up + ACT exp-table preload during the
            # input-DMA window. Dummy matmuls keep the PE HAM busy while
            # xT/w DMAs land, so real matmuls start at 2.4 GHz.
            dumA = cpool.tile([128, 128], f16, tag="dumA", name="dumA")
            dumB = cpool.tile([128, 512], f16, tag="dumB", name="dumB")
            dumE = cpool.tile([128, 64], f16, tag="dumE", name="dumE")
            nc.gpsimd.memset(dumA[:], 0.0)
            nc.gpsimd.memset(dumB[:], 0.0)
            # exp-table preload so the first real exp doesn't pay the
            # ~1.3us ACT_TABLE_LOAD
            nc.scalar.activation(dumE[:], dumB[:, 0:64], Exp)
            for i in range(NWARM):
                wps = mm_ps.tile([128, IB], f32, tag="ps", name=f"warm{i}")
                nc.tensor.matmul(wps[:], dumA[:], dumB[:], start=True, stop=True)

            copy_count = [0]

            def psum_copy(dst, src):
                # PSUM->SBUF drains: 1-in-3 on ScalarE, rest on VectorE -
                # ScalarE's exp stream leaves it less headroom than the DVE.
                copy_count[0] += 1
                if copy_count[0] % 3 == 0:
                    nc.scalar.copy(dst, src)
                else:
                    nc.vector.tensor_copy(dst, src)

            def qk_proj_mms(t, ib, which, copy_eng=None):
                # one Q^T (which=0) or K^T (which=1) projection group as a
                # list of single-matmul closures + final copy closure, so the
                # scheduler can spread them across attention steps.
                dst = QT_sb if which == 0 else KT_sb
                col0 = 0 if which == 0 else D
                ps_box = []

                def mm(k):
                    def go():
                        if k == 0:
                            ps_box.append(
                                mm_ps.tile([128, IB], f32, tag="ps", name=f"qkp{t}{ib}{which}")
                            )
                        nc.tensor.matmul(
                            ps_box[0][:],
                            w_sb[k][:, col0 + t * 128 : col0 + (t + 1) * 128],
                            xT_sb[k][:, ib * IB : (ib + 1) * IB],
                            start=(k == 0),
                            stop=(k == KT - 1),
                        )
                        if k == KT - 1:
                            d = dst[t][:, ib * IB : (ib + 1) * IB]
                            if copy_eng == "scalar":
                                nc.scalar.copy(d, ps_box[0][:])
                            elif copy_eng == "vector":
                                nc.vector.tensor_copy(d, ps_box[0][:])
                            else:
                                psum_copy(d, ps_box[0][:])
                    return go

                return [mm(k) for k in range(KT)]

            def v_proj_mms(nt):
                ps_box = []

                def mm(k):
                    def go():
                        if k == 0:
                            ps_box.append(
                                mm_ps.tile([128, D], f32, tag="ps", name=f"vps{nt}")
                            )
                        nc.tensor.matmul(
                            ps_box[0][:],
                            xT_sb[k][:, nt * 128 : (nt + 1) * 128],
                            w_sb[k][:, 2 * D : 3 * D],
                            start=(k == 0),
                            stop=(k == KT - 1),
                        )
                        if k == KT - 1:
                            nc.vector.tensor_copy(
                                V1_sb[nt].rearrange("p (h c) -> p h c", h=H)[:, :, DH : 2 * DH],
                                ps_box[0].rearrange("p (h c) -> p h c", h=H)[:, :, :],
                            )
                    return go

                return [mm(k) for k in range(KT)]

            def out_proj_mms(nt, pool=None, tag="ps"):
                ps_box = []
                pool_ = pool if pool is not None else mm_ps

                def mm(k):
                    def go():
                        if k == 0:
                            ps_box.append(
                                pool_.tile([128, D], f32, tag=tag, name=f"ops{nt}")
                            )
                        nc.tensor.matmul(
                            ps_box[0][:],
                            Utn_sb[k][nt // 4][:, (nt % 4) * 128 : (nt % 4 + 1) * 128],
                            wout_sb[k][:],
                            start=(k == 0),
                            stop=(k == KT - 1),
                        )
                        if k == KT - 1:
                            osb = opool.tile([128, D], f32, tag="osb", name=f"osb{nt}")
                            psum_copy(osb[:], ps_box[0][:])
                            nc.sync.dma_start(out[nt * 128 : (nt + 1) * 128, :], osb[:])
                    return go

                return [mm(k) for k in range(KT)]

            def run_group(mms):
                for fn in mms:
                    fn()

            # Minimal pre-attention work: only what block (ib0, t0) needs up
            # front. Everything else is interleaved at scheduled (block, jt)
            # slots, ~2 matmuls per slot, so it rides in the PE's idle time
            # while ScalarE streams exps.
            run_group(qk_proj_mms(0, 0, 0))  # QT[t0] i-cols 0:512
            run_group(qk_proj_mms(0, 0, 1))  # KT[t0] j-cols 0:512
            run_group(qk_proj_mms(0, 1, 1))  # KT[t0] j-cols 512:1024
            run_group(v_proj_mms(0))
            run_group(v_proj_mms(1))
            run_group(v_proj_mms(2))

            blocks = [(ib, t) for ib in range(NIB) for t in range(KT)]
            # tasks[(bi, jt)] = list of closures (individual matmuls/copies)
            tasks = {}

            def sched(bi, jt, mms, per_slot=2):
                # spread a group's matmuls over consecutive jt slots,
                # per_slot per slot starting at (bi, jt).  NOTE: a group
                # consumed by the next block's pre-issued QK must fully land
                # by slot (bi, 6) - slot (bi, 7)'s tasks are emitted after
                # the pre_qk for block bi+1.
                for i, fn in enumerate(mms):
                    slot = jt + i // per_slot
                    b2, j2 = bi + slot // NJT, slot % NJT
                    tasks.setdefault((b2, j2), []).append(fn)

            # remaining V projections in block 0 (PE has slack there while
            # the exp chain ramps): v_proj(nt) complete before AV(jt=nt)
            for nt in range(3, NJT):
                sched(0, nt - 3, v_proj_mms(nt), per_slot=4)
            # Q^T/K^T ib0 for pair tn, finishing before block tn's first QK
            # (pre-issued at (tn-1, jt=7))
            sched(0, 5, qk_proj_mms(1, 0, 0), per_slot=4)
            sched(0, 6, qk_proj_mms(1, 0, 1), per_slot=4)
            for bi, tn in ((1, 2), (2, 3)):
                sched(bi, 3, qk_proj_mms(tn, 0, 0))
                sched(bi, 4, qk_proj_mms(tn, 0, 1))
            # K^T j-cols 512:1024 of pair tn, needed from block tn's jt=4.
            # These groups land in slots 1-2 where the DVE already runs the
            # previous block's norm pairs - pin their PSUM-drain copy to
            # ScalarE so it doesn't head-of-line block the et-multiplies.
            for tn in (1, 2, 3):
                sched(tn, 1, qk_proj_mms(tn, 1, 1, copy_eng="scalar"))
            for bi in range(4):
                # QT i-cols 512:1024 of pair bi, needed from block 4+bi,
                # whose first QK pre-issues at (3+bi, jt=7)
                sched(bi + 1, 5, qk_proj_mms(bi, 1, 0))
            # wout loads on the scalar queue after the first eb tiles (slot
            # (0,5) -> trigger lands behind eb0-3), well before block 5
            def load_wout():
                for k in range(KT):
                    nc.scalar.dma_start(
                        wout_sb[k][:], wout[k * 128 : (k + 1) * 128, :]
                    )

            tasks.setdefault((0, 5), []).insert(0, load_wout)
            # out projections for the ib=0 half: Utn[*][0] ready after block
            # 3's norms (flushed at block 4, jt=1)
            for nt in range(4):
                sched(5 + nt // 2, 2 + 3 * (nt % 2), out_proj_mms(nt))
            # ib=1 half, k-tiles 0..2: Utn[0..2][1] are ready once block 6's
            # norms flush at (7,1) - pre-accumulate nt=4,5 during block 7 so
            # only their k=3 matmul (plus nt=6,7) remains after the final
            # norm.  mm_ps has exactly 2 free slots alongside block 7's ups.
            tail_pre = {nt: out_proj_mms(nt) for nt in (4, 5)}
            for nt in (6, 7):
                # nt=6,7 accumulate in st_ps slots, which free up as block
                # 7's last exps drain - their k=0..2 matmuls fill the PE's
                # tail window while the final norms run
                tail_pre[nt] = out_proj_mms(nt, pool=st_ps, tag="st")
            for i, nt in enumerate((4, 5)):
                sched(7, 2 + 2 * i, tail_pre[nt][:KT - 1], per_slot=2)

            # ---- Phase 2: attention (transposed), even/odd heads paired ----
            # The two heads of pair t sit at partitions 0:64 / 64:128 of
            # QT_sb[t]/KT_sb[t]. One exp / one bias-multiply covers both.
            pending_norms = []

            def flush_norms():
                while pending_norms:
                    pending_norms.pop(0)()

            def make_qk(t, ib):
                def qk(jt):
                    st = st_ps.tile(
                        [128, 2 * IB], f32, bufs=2, tag="st", name=f"st{t}{ib}{jt}"
                    )
                    nc.tensor.matmul(
                        st[:, 0:IB],
                        KT_sb[t][0:64, jt * 128 : (jt + 1) * 128],
                        QT_sb[t][0:64, ib * IB : (ib + 1) * IB],
                        start=True,
                        stop=True,
                    )
                    nc.tensor.matmul(
                        st[:, IB : 2 * IB],
                        KT_sb[t][64:128, jt * 128 : (jt + 1) * 128],
                        QT_sb[t][64:128, ib * IB : (ib + 1) * IB],
                        start=True,
                        stop=True,
                    )
                    return st
                return qk

            pre_qk = None
            for bi, (ib, t) in enumerate(blocks):
                he, ho = 2 * t, 2 * t + 1
                ups_e = mm_ps.tile([128, IB], f32, tag="ps", name=f"upse{t}{ib}")
                ups_o = mm_ps.tile([128, IB], f32, tag="ps", name=f"upso{t}{ib}")
                qk = make_qk(t, ib)

                def make_av(jt, et, ups_e=ups_e, ups_o=ups_o, he=he, ho=ho):
                    def go():
                        nc.tensor.matmul(
                            ups_e[:],
                            V1_sb[jt][:, he * 128 : (he + 1) * 128],
                            et[:, 0:IB],
                            start=(jt == 0),
                            stop=(jt == NJT - 1),
                        )
                        nc.tensor.matmul(
                            ups_o[:],
                            V1_sb[jt][:, ho * 128 : (ho + 1) * 128],
                            et[:, IB : 2 * IB],
                            start=(jt == 0),
                            stop=(jt == NJT - 1),
                        )
                    return go

                # software pipeline: QK(jt+1) issues on PE before AV(jt), and
                # AV(jt) is deferred a full slot (emitted at jt+1, behind the
                # filler tasks) - the PE queue is in-order, so this gives the
                # QK->exp->mul chain ~2 steps of latency budget before an
                # unready et can block the PE.
                sts = [pre_qk] if pre_qk is not None else [qk(0)]
                pre_qk = None
                pend_av = None
                for jt in range(NJT):
                    if jt + 1 < NJT:
                        sts.append(qk(jt + 1))
                    elif bi + 1 < len(blocks):
                        nib, nt_ = blocks[bi + 1]
                        pre_qk = make_qk(nt_, nib)(0)
                    st = sts[jt]
                    eb = load_eb(t, ib, jt)
                    et0 = stream.tile([128, 2 * IB], f16, tag="et0", bufs=5)
                    nc.scalar.activation(et0[:], st[:], Exp)
                    et = stream.tile([128, 2 * IB], f16, tag="et", bufs=5)
                    nc.vector.tensor_mul(et[:], et0[:], eb[:])
                    if jt in (1, 2) and pending_norms:
                        # one half of the previous block's norms per slot,
                        # deferred + split so the DVE never sees a >1.5us
                        # burst between this block's et-multiplies
                        pending_norms.pop(0)()
                    for fn in tasks.get((bi, jt), ()):
                        fn()
                    if pend_av is not None:
                        pend_av()
                    pend_av = make_av(jt, et)
                if bi + 1 < len(blocks):
                    pend_av()
                else:
                    # last block: slot the nt=6,7 out-projection partials
                    # around the final AV so the PE tail window stays full
                    for fn in tail_pre[6][: KT - 1]:
                        fn()
                    pend_av()
                    for fn in tail_pre[7][: KT - 1]:
                        fn()

                def make_norm(po, ups, t=t, ib=ib):
                    def go():
                        rb = stream.tile(
                            [64, IB], f32, tag="rb", name=f"rb{t}{ib}{po}"
                        )
                        nc.vector.reciprocal_approx_fast(rb[:, :], ups[0:64, :])
                        nc.vector.tensor_mul(
                            Utn_sb[t][ib][po : po + 64, :],
                            ups[64:128, :],
                            rb[:, :],
                        )
                    return go

                pending_norms.append(make_norm(0, ups_e))
                pending_norms.append(make_norm(64, ups_o))
            flush_norms()

            # ---- Phase 3: final k-tile of each remaining output projection ----
            for nt in (4, 5, 6, 7):
                run_group(tail_pre[nt][KT - 1 :])

    return nc


def _get_graph():
    if "nc" not in _CACHE:
        nc = _build_graph()
        nc.compile()
        _CACHE["nc"] = nc
    return _CACHE["nc"]


def _prep_inputs(x, pos_bias, w_qkv, w_out):
    x = np.asarray(x, dtype=np.float32)
    pos_bias = np.asarray(pos_bias, dtype=np.float32)
    w_qkv = np.asarray(w_qkv, dtype=np.float32)
    w_out = np.asarray(w_out, dtype=np.float32)

    wqkv_mod = w_qkv.copy()
    wqkv_mod[:, :D] *= SCALE
    wout16 = w_out.astype(np.float16)
    wqkv16 = wqkv_mod.astype(np.float16)
    # prepacked exp(bias^T) tiles: ebt[t, ib, jt] = [128 j, he-i | ho-i]
    ebt = np.exp(pos_bias.transpose(0, 2, 1)).astype(np.float16)  # [h, j, i]
    ebt4 = ebt.reshape(KT, 2, NJT, 128, NIB, IB)  # [t, par, jt, p, ib, i]
    ebt_tiles = np.ascontiguousarray(
        ebt4.transpose(0, 4, 2, 3, 1, 5).reshape(KT, NIB, NJT, 128, 2 * IB)
    )

    in_maps = []
    for b in range(NCORES):
        in_maps.append(
            {
                "xT": np.ascontiguousarray(x[b].T.astype(np.float16)),
                "wqkv": wqkv16,
                "wout": wout16,
                "ebt": ebt_tiles,
            }
        )
    return in_maps


def _run(x, pos_bias, w_qkv, w_out, trace=False):
    from concourse.bass_utils import run_bass_kernel_spmd

    nc = _get_graph()
    in_maps = _prep_inputs(x, pos_bias, w_qkv, w_out)
    res = run_bass_kernel_spmd(
        nc, in_maps, core_ids=list(range(NCORES)), trace=trace
    )
    outs = np.stack([np.asarray(res.results[b]["out"]) for b in range(NCORES)])
    return outs.astype(np.float32), res


def kernel(x, pos_bias, w_qkv, w_out):
    outs, _ = _run(x, pos_bias, w_qkv, w_out, trace=False)
    return outs


# revision 34
# speedup vs baseline: 1.0490x; 1.0192x over previous
"""Distributed Trainium2 kernel for batched multi-head self-attention with
positional bias.

Reference computation (per batch element b):
    qkv = x[b] @ w_qkv ; split into q,k,v ; heads of 64
    sim = (q * 64**-0.5) @ k^T + pos_bias          # [h, n, n]
    attn = softmax(sim, axis=-1)
    out[b] = (attn @ v).reshape(n, hidden) @ w_out

Sharding: pure data-parallel - core i computes batch element i (B == 8 ==
n_cores), no collectives.

Device algorithm (per core), designed to avoid all on-chip transposes:
  - host supplies xT = x[b].T, so projections produce Q^T,K^T ([d, n]) and V
    ([n, d]) directly with natural-layout matmuls.
  - attention is computed transposed: St[j,i] = sum_d K^T[d,j] Q^T[d,i];
    softmax over j is handled via exp (ScalarE) * exp(bias^T) (host
    precomputed, fp16, prepacked per-tile) and a ones-block in the AV
    matmul's stationary operand, which makes PSUM rows 0:64 the softmax
    denominators.
  - U''[64:128] * 1/U''[0:64] gives the normalized per-head context, already
    in the [hidden, n] layout the output projection needs as lhsT.

Scheduling (v2): fine-grained input DMAs split across the sync and scalar
HW-DGE queues so the first projections and first bias tiles land ~5us
earlier; projection matmuls are spread ~2 per attention step as PE gap
filler (the exp->mul chain latency otherwise stalls the AV matmuls); PSUM
drains alternate ScalarE/VectorE.
"""

import numpy as np

B, N, D = 8, 1024, 512
H, DH = 8, 64
SCALE = DH**-0.5
NCORES = 8
KT = D // 128  # 4 k-tiles over model dim / hidden dim
NJT = N // 128  # 8 j-tiles
IB = 512
NIB = N // IB  # 2 i-blocks
NWARM = 12

_CACHE = {}


def _build_graph(sim=False):
    import concourse.bass as bass
    import concourse.mybir as mybir
    from concourse import tile

    f32 = mybir.dt.float32
    f16 = mybir.dt.float16
    Exp = mybir.ActivationFunctionType.Exp

    import concourse.bacc as bacc

    # target_bir_lowering=False: bass/bacc lower to per-engine streams with
    # standalone waits itself; walrus's sync structs hold few waits and
    # reject Tile-generated multi-wait instructions otherwise.
    nc = bacc.Bacc(None, target_bir_lowering=False, debug=False)
    xT = nc.declare_dram_parameter("xT", [D, N], f16, isOutput=False)
    wqkv = nc.declare_dram_parameter("wqkv", [D, 3 * D], f16, isOutput=False)
    wout = nc.declare_dram_parameter("wout", [D, D], f16, isOutput=False)
    # host-prepacked exp(bias^T) tiles: ebt[t, ib, jt] = [128 j, he-i | ho-i]
    ebt = nc.declare_dram_parameter(
        "ebt", [KT, NIB, NJT, 128, 2 * IB], f16, isOutput=False
    )
    out = nc.declare_dram_parameter("out", [N, D], f32, isOutput=True)

    with tile.TileContext(nc) as tc:
        with (
            tc.tile_pool(name="const", bufs=1) as cpool,
            tc.tile_pool(name="mm_ps", bufs=4, space="PSUM") as mm_ps,
            tc.tile_pool(name="st_ps", bufs=2, space="PSUM") as st_ps,
            tc.tile_pool(name="stream", bufs=4) as stream,
            tc.tile_pool(name="osb", bufs=4) as opool,
        ):
            # ---- Phase 0: resident allocation + fine-grained loads ----
            # sync queue: xT k-tiles, then w q-cols, then w k-cols, then the
            # steady eb stream.  scalar queue (idle engine at start): w
            # v-cols, the first 4 eb tiles, and wout - all off the critical
            # path of the first Q/K projections.
            w_sb = [
                cpool.tile([128, 3 * D], f16, tag=f"w{k}", name=f"w{k}")
                for k in range(KT)
            ]
            xT_sb = [
                cpool.tile([128, N], f16, tag=f"xt{k}", name=f"xt{k}")
                for k in range(KT)
            ]
            wout_sb = [
                cpool.tile([128, D], f16, tag=f"wo{k}", name=f"wo{k}")
                for k in range(KT)
            ]
            # xT is split across both queues so the 1.25MB critical startup
            # set (xT + the t=0 column slices of w_q / w_k, which gate the
            # first Q/K projections) finishes at the HBM-bus floor; the
            # t=1..3 w slices follow and are needed ~10us later
            for k in (0, 1):
                nc.sync.dma_start(xT_sb[k][:], xT[k * 128 : (k + 1) * 128, :])
            for k in (2, 3):
                nc.scalar.dma_start(xT_sb[k][:], xT[k * 128 : (k + 1) * 128, :])
            for c0, c1 in ((0, 128), (D, D + 128), (128, D), (D + 128, 2 * D)):
                for k in range(KT):
                    nc.sync.dma_start(
                        w_sb[k][:, c0:c1], wqkv[k * 128 : (k + 1) * 128, c0:c1]
                    )
            # v-cols first on the scalar queue: v_proj(0..2) run right after
            # the three upfront q/k projection groups
            for k in range(KT):
                nc.scalar.dma_start(
                    w_sb[k][:, 2 * D : 3 * D],
                    wqkv[k * 128 : (k + 1) * 128, 2 * D : 3 * D],
                )

            # V1: per jt a [128, H*128] tensor holding, per head, the AV
            # stationary operand [ones | v_h] (ones via memset, v written by
            # the V projection).
            V1_sb = []
            for jt in range(NJT):
                v1 = cpool.tile([128, H * 128], f16, tag=f"v1_{jt}", name=f"v1_{jt}")
                nc.gpsimd.memset(v1[:], 1.0)
                V1_sb.append(v1)

            QT_sb = [cpool.tile([128, N], f16, tag=f"qt{t}", name=f"qt{t}") for t in range(KT)]
            KT_sb = [cpool.tile([128, N], f16, tag=f"kt{t}", name=f"kt{t}") for t in range(KT)]
            Utn_sb = [
                [
                    cpool.tile([128, IB], f16, tag=f"ut{t}_{ib}", name=f"ut{t}_{ib}")
                    for ib in range(NIB)
                ]
                for t in range(KT)
            ]

            # eb DMA launcher: first block's tiles ride the scalar queue
            # (issued during startup), everything else the sync queue.
            eb_count = [0]

            def load_eb(t, ib, jt):
                eb = stream.tile([128, 2 * IB], f16, tag="eb", bufs=6)
                eng = nc.scalar if eb_count[0] < 4 else nc.sync
                eb_count[0] += 1
                eng.dma_start(eb[:], ebt[t, ib, jt, :, :])
                return eb

            # ---- Phase 0b: PE warm-up + ACT exp-table preload during the
            # input-DMA window. Dummy matmuls keep the PE HAM busy while
            # xT/w DMAs land, so real matmuls start at 2.4 GHz.
            dumA = cpool.tile([128, 128], f16, tag="dumA", name="dumA")
            dumB = cpool.tile([128, 512], f16, tag="dumB", name="dumB")
            dumE = cpool.tile([128, 64], f16, tag="dumE", name="dumE")
            nc.gpsimd.memset(dumA[:], 0.0)
            nc.gpsimd.memset(dumB[:], 0.0)
            # exp-table preload so the first real exp doesn't pay the
            # ~1.3us ACT_TABLE_LOAD
            nc.scalar.activation(dumE[:], dumB[:, 0:64], Exp)
            for i in range(NWARM):
                wps = mm_ps.tile([128, IB], f32, tag="ps", name=f"warm{i}")
                nc.tensor.matmul(wps[:], dumA[:], dumB[:], start=True, stop=True)

            copy_count = [0]

            def psum_copy(dst, src):
                # PSUM->SBUF drains: 1-in-3 on ScalarE, rest on VectorE -
                # ScalarE's exp stream leaves it less headroom than the DVE.
                copy_count[0] += 1
                if copy_count[0] % 3 == 0:
                    nc.scalar.copy(dst, src)
                else:
                    nc.vector.tensor_copy(dst, src)

            def qk_proj_mms(t, ib, which, copy_eng=None):
                # one Q^T (which=0) or K^T (which=1) projection group as a
                # list of single-matmul closures + final copy closure, so the
                # scheduler can spread them across attention steps.
                dst = QT_sb if which == 0 else KT_sb
                col0 = 0 if which == 0 else D
                ps_box = []

                def mm(k):
                    def go():
                        if k == 0:
                            ps_box.append(
                                mm_ps.tile([128, IB], f32, tag="ps", name=f"qkp{t}{ib}{which}")
                            )
                        nc.tensor.matmul(
                            ps_box[0][:],
                            w_sb[k][:, col0 + t * 128 : col0 + (t + 1) * 128],
                            xT_sb[k][:, ib * IB : (ib + 1) * IB],
                            start=(k == 0),
                            stop=(k == KT - 1),
                        )
                        if k == KT - 1:
                            d = dst[t][:, ib * IB : (ib + 1) * IB]
                            if copy_eng == "scalar":
                                nc.scalar.copy(d, ps_box[0][:])
                            elif copy_eng == "vector":
                                nc.vector.tensor_copy(d, ps_box[0][:])
                            else:
                                psum_copy(d, ps_box[0][:])
                    return go

                return [mm(k) for k in range(KT)]

            def v_proj_mms(nt):
                ps_box = []

                def mm(k):
                    def go():
                        if k == 0:
                            ps_box.append(
                                mm_ps.tile([128, D], f32, tag="ps", name=f"vps{nt}")
                            )
                        nc.tensor.matmul(
                            ps_box[0][:],
                            xT_sb[k][:, nt * 128 : (nt + 1) * 128],
                            w_sb[k][:, 2 * D : 3 * D],
                            start=(k == 0),
                            stop=(k == KT - 1),
                        )
                        if k == KT - 1:
                            nc.vector.tensor_copy(
                                V1_sb[nt].rearrange("p (h c) -> p h c", h=H)[:, :, DH : 2 * DH],
                                ps_box[0].rearrange("p (h c) -> p h c", h=H)[:, :, :],
                            )
                    return go

                return [mm(k) for k in range(KT)]

            out_ps_boxes = {}

            def out_proj_mms(nt, pool=None, tag="ps"):
                ps_box = []
                out_ps_boxes[nt] = ps_box
                pool_ = pool if pool is not None else mm_ps

                def mm(k):
                    def go():
                        if k == 0:
                            ps_box.append(
                                pool_.tile([128, D], f32, tag=tag, name=f"ops{nt}")
                            )
                        nc.tensor.matmul(
                            ps_box[0][:],
                            Utn_sb[k][nt // 4][:, (nt % 4) * 128 : (nt % 4 + 1) * 128],
                            wout_sb[k][:],
                            start=(k == 0),
                            stop=(k == KT - 1),
                        )
                        if k == KT - 1:
                            osb = opool.tile([128, D], f32, tag="osb", name=f"osb{nt}")
                            psum_copy(osb[:], ps_box[0][:])
                            nc.sync.dma_start(out[nt * 128 : (nt + 1) * 128, :], osb[:])
                    return go

                return [mm(k) for k in range(KT)]

            def run_group(mms):
                for fn in mms:
                    fn()

            # Minimal pre-attention work: only what block (ib0, t0) needs up
            # front. Everything else is interleaved at scheduled (block, jt)
            # slots, ~2 matmuls per slot, so it rides in the PE's idle time
            # while ScalarE streams exps.
            run_group(qk_proj_mms(0, 0, 0))  # QT[t0] i-cols 0:512
            run_group(qk_proj_mms(0, 0, 1))  # KT[t0] j-cols 0:512
            run_group(qk_proj_mms(0, 1, 1))  # KT[t0] j-cols 512:1024
            run_group(v_proj_mms(0))
            run_group(v_proj_mms(1))
            run_group(v_proj_mms(2))

            blocks = [(ib, t) for ib in range(NIB) for t in range(KT)]
            # tasks[(bi, jt)] = list of closures (individual matmuls/copies)
            tasks = {}

            def sched(bi, jt, mms, per_slot=2):
                # spread a group's matmuls over consecutive jt slots,
                # per_slot per slot starting at (bi, jt).  NOTE: a group
                # consumed by the next block's pre-issued QK must fully land
                # by slot (bi, 6) - slot (bi, 7)'s tasks are emitted after
                # the pre_qk for block bi+1.
                for i, fn in enumerate(mms):
                    slot = jt + i // per_slot
                    b2, j2 = bi + slot // NJT, slot % NJT
                    tasks.setdefault((b2, j2), []).append(fn)

            # remaining V projections in block 0 (PE has slack there while
            # the exp chain ramps): v_proj(nt) complete before AV(jt=nt)
            for nt in range(3, NJT):
                sched(0, nt - 3, v_proj_mms(nt), per_slot=4)
            # Q^T/K^T ib0 for pair tn, finishing before block tn's first QK
            # (pre-issued at (tn-1, jt=7))
            sched(0, 5, qk_proj_mms(1, 0, 0), per_slot=4)
            sched(0, 6, qk_proj_mms(1, 0, 1), per_slot=4)
            for bi, tn in ((1, 2), (2, 3)):
                sched(bi, 3, qk_proj_mms(tn, 0, 0))
                sched(bi, 4, qk_proj_mms(tn, 0, 1))
            # K^T j-cols 512:1024 of pair tn, needed from block tn's jt=4.
            # These groups land in slots 1-2 where the DVE already runs the
            # previous block's norm pairs - pin their PSUM-drain copy to
            # ScalarE so it doesn't head-of-line block the et-multiplies.
            for tn in (1, 2, 3):
                sched(tn, 1, qk_proj_mms(tn, 1, 1, copy_eng="scalar"))
            for bi in range(4):
                # QT i-cols 512:1024 of pair bi, needed from block 4+bi,
                # whose first QK pre-issues at (3+bi, jt=7)
                sched(bi + 1, 5, qk_proj_mms(bi, 1, 0))
            # wout loads on the scalar queue after the first eb tiles (slot
            # (0,5) -> trigger lands behind eb0-3), well before block 5
            def load_wout():
                for k in range(KT):
                    nc.scalar.dma_start(
                        wout_sb[k][:], wout[k * 128 : (k + 1) * 128, :]
                    )

            tasks.setdefault((0, 5), []).insert(0, load_wout)
            # out projections for the ib=0 half: Utn[*][0] ready after block
            # 3's norms (flushed at block 4, jt=1)
            for nt in range(4):
                sched(5 + nt // 2, 2 + 3 * (nt % 2), out_proj_mms(nt))
            # ib=1 half, k-tiles 0..2: Utn[0..2][1] are ready once block 6's
            # norms flush at (7,1) - pre-accumulate nt=4,5 during block 7 so
            # only their k=3 matmul (plus nt=6,7) remains after the final
            # norm.  mm_ps has exactly 2 free slots alongside block 7's ups.
            tail_pre = {nt: out_proj_mms(nt) for nt in (4, 5)}
            for nt in (6, 7):
                # nt=6,7 accumulate in st_ps slots, which free up as block
                # 7's last exps drain - their k=0..2 matmuls fill the PE's
                # tail window while the final norms run
                tail_pre[nt] = out_proj_mms(nt, pool=st_ps, tag="st")
            for i, nt in enumerate((4, 5)):
                sched(7, 2 + 2 * i, tail_pre[nt][:KT - 1], per_slot=2)

            # ---- Phase 2: attention (transposed), even/odd heads paired ----
            # The two heads of pair t sit at partitions 0:64 / 64:128 of
            # QT_sb[t]/KT_sb[t]. One exp / one bias-multiply covers both.
            pending_norms = []

            def flush_norms():
                while pending_norms:
                    pending_norms.pop(0)()

            def make_qk(t, ib):
                def qk(jt):
                    st = st_ps.tile(
                        [128, 2 * IB], f32, bufs=2, tag="st", name=f"st{t}{ib}{jt}"
                    )
                    nc.tensor.matmul(
                        st[:, 0:IB],
                        KT_sb[t][0:64, jt * 128 : (jt + 1) * 128],
                        QT_sb[t][0:64, ib * IB : (ib + 1) * IB],
                        start=True,
                        stop=True,
                    )
                    nc.tensor.matmul(
                        st[:, IB : 2 * IB],
                        KT_sb[t][64:128, jt * 128 : (jt + 1) * 128],
                        QT_sb[t][64:128, ib * IB : (ib + 1) * IB],
                        start=True,
                        stop=True,
                    )
                    return st
                return qk

            pre_qk = None
            for bi, (ib, t) in enumerate(blocks):
                he, ho = 2 * t, 2 * t + 1
                ups_e = mm_ps.tile([128, IB], f32, tag="ps", name=f"upse{t}{ib}")
                ups_o = mm_ps.tile([128, IB], f32, tag="ps", name=f"upso{t}{ib}")
                qk = make_qk(t, ib)

                def make_av(jt, et, ups_e=ups_e, ups_o=ups_o, he=he, ho=ho):
                    def go():
                        nc.tensor.matmul(
                            ups_e[:],
                            V1_sb[jt][:, he * 128 : (he + 1) * 128],
                            et[:, 0:IB],
                            start=(jt == 0),
                            stop=(jt == NJT - 1),
                        )
                        nc.tensor.matmul(
                            ups_o[:],
                            V1_sb[jt][:, ho * 128 : (ho + 1) * 128],
                            et[:, IB : 2 * IB],
                            start=(jt == 0),
                            stop=(jt == NJT - 1),
                        )
                    return go

                # software pipeline: QK(jt+1) issues on PE before AV(jt), and
                # AV(jt) is deferred a full slot (emitted at jt+1, behind the
                # filler tasks) - the PE queue is in-order, so this gives the
                # QK->exp->mul chain ~2 steps of latency budget before an
                # unready et can block the PE.
                sts = [pre_qk] if pre_qk is not None else [qk(0)]
                pre_qk = None
                pend_av = None
                for jt in range(NJT):
                    if jt + 1 < NJT:
                        sts.append(qk(jt + 1))
                    elif bi + 1 < len(blocks):
                        nib, nt_ = blocks[bi + 1]
                        pre_qk = make_qk(nt_, nib)(0)
                    st = sts[jt]
                    eb = load_eb(t, ib, jt)
                    et0 = stream.tile([128, 2 * IB], f16, tag="et0", bufs=5)
                    nc.scalar.activation(et0[:], st[:], Exp)
                    et = stream.tile([128, 2 * IB], f16, tag="et", bufs=5)
                    nc.vector.tensor_mul(et[:], et0[:], eb[:])
                    if jt in (1, 2) and pending_norms:
                        # one half of the previous block's norms per slot,
                        # deferred + split so the DVE never sees a >1.5us
                        # burst between this block's et-multiplies
                        pending_norms.pop(0)()
                    for fn in tasks.get((bi, jt), ()):
                        fn()
                    if pend_av is not None:
                        pend_av()
                    pend_av = make_av(jt, et)
                if bi + 1 < len(blocks):
                    pend_av()
                else:
                    # last block: slot the nt=6,7 out-projection partials
                    # around the final AV so the PE tail window stays full
                    for fn in tail_pre[6][: KT - 1]:
                        fn()
                    pend_av()
                    for fn in tail_pre[7][: KT - 1]:
                        fn()

                def make_norm(po, ups, t=t, ib=ib):
                    def go():
                        rb = stream.tile(
                            [64, IB], f32, tag="rb", name=f"rb{t}{ib}{po}"
                        )
                        nc.vector.reciprocal_approx_fast(rb[:, :], ups[0:64, :])
                        nc.vector.tensor_mul(
                            Utn_sb[t][ib][po : po + 64, :],
                            ups[64:128, :],
                            rb[:, :],
                        )
                    return go

                pending_norms.append(make_norm(0, ups_e))
                pending_norms.append(make_norm(64, ups_o))
            flush_norms()

            # ---- Phase 3: final k-tile of each remaining output projection.
            # The k=3 matmul is split per head-half so the rows-0:64 halves
            # run right after nmul_e of the last norm, overlapping nmul_o;
            # the final DMAs alternate sync/scalar queues to halve the
            # serialized tail transfer time.
            for lo in (0, 64):
                for nt in (4, 5, 6, 7):
                    ps = out_ps_boxes[nt][0]
                    nc.tensor.matmul(
                        ps[:],
                        Utn_sb[KT - 1][1][lo : lo + 64, (nt % 4) * 128 : (nt % 4 + 1) * 128],
                        wout_sb[KT - 1][lo : lo + 64, :],
                        start=False,
                        stop=(lo == 64),
                    )
            for nt in (4, 5, 6, 7):
                osb = opool.tile([128, D], f32, tag="osb", name=f"osb{nt}")
                psum_copy(osb[:], out_ps_boxes[nt][0][:])
                eng = nc.sync if nt % 2 == 0 else nc.scalar
                eng.dma_start(out[nt * 128 : (nt + 1) * 128, :], osb[:])

    return nc


def _get_graph():
    if "nc" not in _CACHE:
        nc = _build_graph()
        nc.compile()
        _CACHE["nc"] = nc
    return _CACHE["nc"]


def _prep_inputs(x, pos_bias, w_qkv, w_out):
    x = np.asarray(x, dtype=np.float32)
    pos_bias = np.asarray(pos_bias, dtype=np.float32)
    w_qkv = np.asarray(w_qkv, dtype=np.float32)
    w_out = np.asarray(w_out, dtype=np.float32)

    wqkv_mod = w_qkv.copy()
    wqkv_mod[:, :D] *= SCALE
    wout16 = w_out.astype(np.float16)
    wqkv16 = wqkv_mod.astype(np.float16)
    # prepacked exp(bias^T) tiles: ebt[t, ib, jt] = [128 j, he-i | ho-i]
    ebt = np.exp(pos_bias.transpose(0, 2, 1)).astype(np.float16)  # [h, j, i]
    ebt4 = ebt.reshape(KT, 2, NJT, 128, NIB, IB)  # [t, par, jt, p, ib, i]
    ebt_tiles = np.ascontiguousarray(
        ebt4.transpose(0, 4, 2, 3, 1, 5).reshape(KT, NIB, NJT, 128, 2 * IB)
    )

    in_maps = []
    for b in range(NCORES):
        in_maps.append(
            {
                "xT": np.ascontiguousarray(x[b].T.astype(np.float16)),
                "wqkv": wqkv16,
                "wout": wout16,
                "ebt": ebt_tiles,
            }
        )
    return in_maps


def _run(x, pos_bias, w_qkv, w_out, trace=False):
    from concourse.bass_utils import run_bass_kernel_spmd

    nc = _get_graph()
    in_maps = _prep_inputs(x, pos_bias, w_qkv, w_out)
    res = run_bass_kernel_spmd(
        nc, in_maps, core_ids=list(range(NCORES)), trace=trace
    )
    outs = np.stack([np.asarray(res.results[b]["out"]) for b in range(NCORES)])
    return outs.astype(np.float32), res


def kernel(x, pos_bias, w_qkv, w_out):
    outs, _ = _run(x, pos_bias, w_qkv, w_out, trace=False)
    return outs


# revision 42
# speedup vs baseline: 1.2207x; 1.1637x over previous
"""Distributed Trainium2 kernel for batched multi-head self-attention with
positional bias.

Reference computation (per batch element b):
    qkv = x[b] @ w_qkv ; split into q,k,v ; heads of 64
    sim = (q * 64**-0.5) @ k^T + pos_bias          # [h, n, n]
    attn = softmax(sim, axis=-1)
    out[b] = (attn @ v).reshape(n, hidden) @ w_out

Sharding: pure data-parallel - core i computes batch element i (B == 8 ==
n_cores), no collectives.

Device algorithm (per core), designed to avoid all on-chip transposes:
  - host supplies xT = x[b].T, so projections produce Q^T,K^T ([d, n]) and V
    ([n, d]) directly with natural-layout matmuls.
  - attention is computed transposed: St[j,i] = sum_d K^T[d,j] Q^T[d,i];
    softmax over j is handled via exp (ScalarE) * exp(bias^T) (host
    precomputed, fp16, prepacked per-tile) and a ones-block in the AV
    matmul's stationary operand, which makes PSUM rows 0:64 the softmax
    denominators.
  - U''[64:128] * 1/U''[0:64] gives the normalized per-head context, already
    in the [hidden, n] layout the output projection needs as lhsT.

Scheduling (v2): fine-grained input DMAs split across the sync and scalar
HW-DGE queues so the first projections and first bias tiles land ~5us
earlier; projection matmuls are spread ~2 per attention step as PE gap
filler (the exp->mul chain latency otherwise stalls the AV matmuls); PSUM
drains alternate ScalarE/VectorE.
"""

import numpy as np

B, N, D = 8, 1024, 512
H, DH = 8, 64
SCALE = DH**-0.5
NCORES = 8
KT = D // 128  # 4 k-tiles over model dim / hidden dim
NJT = N // 128  # 8 j-tiles
IB = 512
NIB = N // IB  # 2 i-blocks
NWARM = 12

_CACHE = {}


def _build_graph(sim=False):
    import concourse.bass as bass
    import concourse.mybir as mybir
    from concourse import tile

    f32 = mybir.dt.float32
    f16 = mybir.dt.float16
    Exp = mybir.ActivationFunctionType.Exp

    import concourse.bacc as bacc

    # target_bir_lowering=False: bass/bacc lower to per-engine streams with
    # standalone waits itself; walrus's sync structs hold few waits and
    # reject Tile-generated multi-wait instructions otherwise.
    nc = bacc.Bacc(None, target_bir_lowering=False, debug=False)
    xT = nc.declare_dram_parameter("xT", [D, N], f16, isOutput=False)
    wqkv = nc.declare_dram_parameter("wqkv", [D, 3 * D], f16, isOutput=False)
    wout = nc.declare_dram_parameter("wout", [D, D], f16, isOutput=False)
    # host-prepacked exp(bias^T) tiles: ebt[t, ib, jt] = [128 j, he-i | ho-i]
    ebt = nc.declare_dram_parameter(
        "ebt", [KT, NIB, NJT, 128, 2 * IB], f16, isOutput=False
    )
    out = nc.declare_dram_parameter("out", [N, D], f32, isOutput=True)

    with tile.TileContext(nc) as tc:
        with (
            tc.tile_pool(name="const", bufs=1) as cpool,
            tc.tile_pool(name="mm_ps", bufs=4, space="PSUM") as mm_ps,
            tc.tile_pool(name="st_ps", bufs=2, space="PSUM") as st_ps,
            tc.tile_pool(name="stream", bufs=4) as stream,
            tc.tile_pool(name="osb", bufs=4) as opool,
        ):
            # ---- Phase 0: resident allocation + coalesced loads ----
            # Startup is bound by DMA-trigger serialization (~0.63us per
            # dma_start on the issuing engine), so the inputs are folded
            # over their k-tile dim into a handful of multi-dim DMAs.
            # sync queue: xT, then the t=0 w_q/w_k column slices (which gate
            # the first projections), then the rest.  scalar queue: w
            # v-cols, first-block eb tiles, wout.
            w_all = cpool.tile([128, KT * 3 * D], f16, tag="w", name="w")
            wV = w_all.rearrange("p (k c) -> p k c", k=KT)
            xT_all = cpool.tile([128, KT * N], f16, tag="xt", name="xt")
            xV = xT_all.rearrange("p (k n) -> p k n", k=KT)
            wout_all = cpool.tile([128, KT * D], f16, tag="wo", name="wo")
            woV = wout_all.rearrange("p (k c) -> p k c", k=KT)
            xT_d = xT.rearrange("(k p) n -> p k n", p=128)
            wqkv_d = wqkv.rearrange("(k p) c -> p k c", p=128)
            wout_d = wout.rearrange("(k p) c -> p k c", p=128)

            nc.sync.dma_start(xV[:, :, :], xT_d[:, :, :])
            for c0, c1 in ((0, 128), (D, D + 128), (128, D), (D + 128, 2 * D)):
                nc.sync.dma_start(wV[:, :, c0:c1], wqkv_d[:, :, c0:c1])
            # v-cols first on the scalar queue: v_proj(0..2) run right after
            # the three upfront q/k projection groups
            nc.scalar.dma_start(
                wV[:, :, 2 * D : 3 * D], wqkv_d[:, :, 2 * D : 3 * D]
            )

            # V1: per jt a [128, H*128] tensor holding, per head, the AV
            # stationary operand [ones | v_h] (ones via memset, v written by
            # the V projection).
            V1_sb = []
            for jt in range(NJT):
                v1 = cpool.tile([128, H * 128], f16, tag=f"v1_{jt}", name=f"v1_{jt}")
                nc.gpsimd.memset(v1[:], 1.0)
                V1_sb.append(v1)

            QT_sb = [cpool.tile([128, N], f16, tag=f"qt{t}", name=f"qt{t}") for t in range(KT)]
            KT_sb = [cpool.tile([128, N], f16, tag=f"kt{t}", name=f"kt{t}") for t in range(KT)]
            Utn_sb = [
                [
                    cpool.tile([128, IB], f16, tag=f"ut{t}_{ib}", name=f"ut{t}_{ib}")
                    for ib in range(NIB)
                ]
                for t in range(KT)
            ]

            # eb tiles: the first block's first four ride one coalesced
            # scalar-queue DMA into a resident tile (issued during startup);
            # everything else streams one tile at a time on the sync queue.
            eb0123 = cpool.tile([128, 4 * 2 * IB], f16, tag="eb0", name="eb0")
            eb0v = eb0123.rearrange("p (j c) -> p j c", j=4)
            ebt_first = ebt.rearrange("t i j p c -> t i p j c")
            nc.scalar.dma_start(eb0v[:, :, :], ebt_first[0, 0, :, 0:4, :])

            def load_eb(t, ib, jt):
                if t == 0 and ib == 0 and jt < 4:
                    return eb0v[:, jt, :]
                eb = stream.tile([128, 2 * IB], f16, tag="eb", bufs=6)
                nc.sync.dma_start(eb[:], ebt[t, ib, jt, :, :])
                return eb[:]

            # ---- Phase 0b: PE warm-up + ACT exp-table preload during the
            # input-DMA window. Dummy matmuls keep the PE HAM busy while
            # xT/w DMAs land, so real matmuls start at 2.4 GHz.
            dumA = cpool.tile([128, 128], f16, tag="dumA", name="dumA")
            dumB = cpool.tile([128, 512], f16, tag="dumB", name="dumB")
            dumE = cpool.tile([128, 64], f16, tag="dumE", name="dumE")
            nc.gpsimd.memset(dumA[:], 0.0)
            nc.gpsimd.memset(dumB[:], 0.0)
            # exp-table preload so the first real exp doesn't pay the
            # ~1.3us ACT_TABLE_LOAD
            nc.scalar.activation(dumE[:], dumB[:, 0:64], Exp)
            for i in range(NWARM):
                wps = mm_ps.tile([128, IB], f32, tag="ps", name=f"warm{i}")
                nc.tensor.matmul(wps[:], dumA[:], dumB[:], start=True, stop=True)

            copy_count = [0]

            def psum_copy(dst, src):
                # PSUM->SBUF drains: 1-in-3 on ScalarE, rest on VectorE -
                # ScalarE's exp stream leaves it less headroom than the DVE.
                copy_count[0] += 1
                if copy_count[0] % 3 == 0:
                    nc.scalar.copy(dst, src)
                else:
                    nc.vector.tensor_copy(dst, src)

            def qk_proj_mms(t, ib, which, copy_eng=None):
                # one Q^T (which=0) or K^T (which=1) projection group as a
                # list of single-matmul closures + final copy closure, so the
                # scheduler can spread them across attention steps.
                dst = QT_sb if which == 0 else KT_sb
                col0 = 0 if which == 0 else D
                ps_box = []

                def mm(k):
                    def go():
                        if k == 0:
                            ps_box.append(
                                mm_ps.tile([128, IB], f32, tag="ps", name=f"qkp{t}{ib}{which}")
                            )
                        nc.tensor.matmul(
                            ps_box[0][:],
                            wV[:, k, col0 + t * 128 : col0 + (t + 1) * 128],
                            xV[:, k, ib * IB : (ib + 1) * IB],
                            start=(k == 0),
                            stop=(k == KT - 1),
                        )
                        if k == KT - 1:
                            d = dst[t][:, ib * IB : (ib + 1) * IB]
                            if copy_eng == "scalar":
                                nc.scalar.copy(d, ps_box[0][:])
                            elif copy_eng == "vector":
                                nc.vector.tensor_copy(d, ps_box[0][:])
                            else:
                                psum_copy(d, ps_box[0][:])
                    return go

                return [mm(k) for k in range(KT)]

            def v_proj_mms(nt):
                ps_box = []

                def mm(k):
                    def go():
                        if k == 0:
                            ps_box.append(
                                mm_ps.tile([128, D], f32, tag="ps", name=f"vps{nt}")
                            )
                        nc.tensor.matmul(
                            ps_box[0][:],
                            xV[:, k, nt * 128 : (nt + 1) * 128],
                            wV[:, k, 2 * D : 3 * D],
                            start=(k == 0),
                            stop=(k == KT - 1),
                        )
                        if k == KT - 1:
                            nc.vector.tensor_copy(
                                V1_sb[nt].rearrange("p (h c) -> p h c", h=H)[:, :, DH : 2 * DH],
                                ps_box[0].rearrange("p (h c) -> p h c", h=H)[:, :, :],
                            )
                    return go

                return [mm(k) for k in range(KT)]

            out_ps_boxes = {}

            def out_proj_mms(nt, pool=None, tag="ps"):
                ps_box = []
                out_ps_boxes[nt] = ps_box
                pool_ = pool if pool is not None else mm_ps

                def mm(k):
                    def go():
                        if k == 0:
                            ps_box.append(
                                pool_.tile([128, D], f32, tag=tag, name=f"ops{nt}")
                            )
                        nc.tensor.matmul(
                            ps_box[0][:],
                            Utn_sb[k][nt // 4][:, (nt % 4) * 128 : (nt % 4 + 1) * 128],
                            woV[:, k, :],
                            start=(k == 0),
                            stop=(k == KT - 1),
                        )
                        if k == KT - 1:
                            osb = opool.tile([128, D], f32, tag="osb", name=f"osb{nt}")
                            psum_copy(osb[:], ps_box[0][:])
                            nc.sync.dma_start(out[nt * 128 : (nt + 1) * 128, :], osb[:])
                    return go

                return [mm(k) for k in range(KT)]

            def run_group(mms):
                for fn in mms:
                    fn()

            # Minimal pre-attention work: only what block (ib0, t0) needs up
            # front. Everything else is interleaved at scheduled (block, jt)
            # slots, ~2 matmuls per slot, so it rides in the PE's idle time
            # while ScalarE streams exps.
            run_group(qk_proj_mms(0, 0, 0))  # QT[t0] i-cols 0:512
            run_group(qk_proj_mms(0, 0, 1))  # KT[t0] j-cols 0:512
            run_group(qk_proj_mms(0, 1, 1))  # KT[t0] j-cols 512:1024
            run_group(v_proj_mms(0))
            run_group(v_proj_mms(1))
            run_group(v_proj_mms(2))

            blocks = [(ib, t) for ib in range(NIB) for t in range(KT)]
            # tasks[(bi, jt)] = list of closures (individual matmuls/copies)
            tasks = {}

            def sched(bi, jt, mms, per_slot=2):
                # spread a group's matmuls over consecutive jt slots,
                # per_slot per slot starting at (bi, jt).  NOTE: a group
                # consumed by the next block's pre-issued QK must fully land
                # by slot (bi, 6) - slot (bi, 7)'s tasks are emitted after
                # the pre_qk for block bi+1.
                for i, fn in enumerate(mms):
                    slot = jt + i // per_slot
                    b2, j2 = bi + slot // NJT, slot % NJT
                    tasks.setdefault((b2, j2), []).append(fn)

            # remaining V projections in block 0 (PE has slack there while
            # the exp chain ramps): v_proj(nt) complete before AV(jt=nt)
            for nt in range(3, NJT):
                sched(0, nt - 3, v_proj_mms(nt), per_slot=4)
            # Q^T/K^T ib0 for pair tn, finishing before block tn's first QK
            # (pre-issued at (tn-1, jt=7))
            sched(0, 5, qk_proj_mms(1, 0, 0), per_slot=4)
            sched(0, 6, qk_proj_mms(1, 0, 1), per_slot=4)
            for bi, tn in ((1, 2), (2, 3)):
                sched(bi, 3, qk_proj_mms(tn, 0, 0))
                sched(bi, 4, qk_proj_mms(tn, 0, 1))
            # K^T j-cols 512:1024 of pair tn, needed from block tn's jt=4.
            # These groups land in slots 1-2 where the DVE already runs the
            # previous block's norm pairs - pin their PSUM-drain copy to
            # ScalarE so it doesn't head-of-line block the et-multiplies.
            for tn in (1, 2, 3):
                sched(tn, 1, qk_proj_mms(tn, 1, 1, copy_eng="scalar"))
            for bi in range(4):
                # QT i-cols 512:1024 of pair bi, needed from block 4+bi,
                # whose first QK pre-issues at (3+bi, jt=7)
                sched(bi + 1, 5, qk_proj_mms(bi, 1, 0))
            # wout loads on the scalar queue after the first eb tiles (slot
            # (0,5) -> trigger lands behind eb0-3), well before block 5
            def load_wout():
                nc.scalar.dma_start(woV[:, :, :], wout_d[:, :, :])

            tasks.setdefault((0, 5), []).insert(0, load_wout)
            # out projections for the ib=0 half: Utn[*][0] ready after block
            # 3's norms (flushed at block 4, jt=1)
            for nt in range(4):
                sched(5 + nt // 2, 2 + 3 * (nt % 2), out_proj_mms(nt))
            # ib=1 half, k-tiles 0..2: Utn[0..2][1] are ready once block 6's
            # norms flush at (7,1) - pre-accumulate nt=4,5 during block 7 so
            # only their k=3 matmul (plus nt=6,7) remains after the final
            # norm.  mm_ps has exactly 2 free slots alongside block 7's ups.
            tail_pre = {nt: out_proj_mms(nt) for nt in (4, 5)}
            for nt in (6, 7):
                # nt=6,7 accumulate in st_ps slots, which free up as block
                # 7's last exps drain - their k=0..2 matmuls fill the PE's
                # tail window while the final norms run
                tail_pre[nt] = out_proj_mms(nt, pool=st_ps, tag="st")
            for i, nt in enumerate((4, 5)):
                sched(7, 2 + 2 * i, tail_pre[nt][:KT - 1], per_slot=2)

            # ---- Phase 2: attention (transposed), even/odd heads paired ----
            # The two heads of pair t sit at partitions 0:64 / 64:128 of
            # QT_sb[t]/KT_sb[t]. One exp / one bias-multiply covers both.
            pending_norms = []

            def flush_norms():
                while pending_norms:
                    pending_norms.pop(0)()

            def make_qk(t, ib):
                def qk(jt):
                    st = st_ps.tile(
                        [128, 2 * IB], f32, bufs=2, tag="st", name=f"st{t}{ib}{jt}"
                    )
                    nc.tensor.matmul(
                        st[:, 0:IB],
                        KT_sb[t][0:64, jt * 128 : (jt + 1) * 128],
                        QT_sb[t][0:64, ib * IB : (ib + 1) * IB],
                        start=True,
                        stop=True,
                    )
                    nc.tensor.matmul(
                        st[:, IB : 2 * IB],
                        KT_sb[t][64:128, jt * 128 : (jt + 1) * 128],
                        QT_sb[t][64:128, ib * IB : (ib + 1) * IB],
                        start=True,
                        stop=True,
                    )
                    return st
                return qk

            pre_qk = None
            for bi, (ib, t) in enumerate(blocks):
                he, ho = 2 * t, 2 * t + 1
                ups_e = mm_ps.tile([128, IB], f32, tag="ps", name=f"upse{t}{ib}")
                ups_o = mm_ps.tile([128, IB], f32, tag="ps", name=f"upso{t}{ib}")
                qk = make_qk(t, ib)

                def make_av(jt, et, ups_e=ups_e, ups_o=ups_o, he=he, ho=ho):
                    def go():
                        nc.tensor.matmul(
                            ups_e[:],
                            V1_sb[jt][:, he * 128 : (he + 1) * 128],
                            et[:, 0:IB],
                            start=(jt == 0),
                            stop=(jt == NJT - 1),
                        )
                        nc.tensor.matmul(
                            ups_o[:],
                            V1_sb[jt][:, ho * 128 : (ho + 1) * 128],
                            et[:, IB : 2 * IB],
                            start=(jt == 0),
                            stop=(jt == NJT - 1),
                        )
                    return go

                # software pipeline: QK(jt+1) issues on PE before AV(jt), and
                # AV(jt) is deferred a full slot (emitted at jt+1, behind the
                # filler tasks) - the PE queue is in-order, so this gives the
                # QK->exp->mul chain ~2 steps of latency budget before an
                # unready et can block the PE.
                sts = [pre_qk] if pre_qk is not None else [qk(0)]
                pre_qk = None
                pend_av = None
                for jt in range(NJT):
                    if jt + 1 < NJT:
                        sts.append(qk(jt + 1))
                    elif bi + 1 < len(blocks):
                        nib, nt_ = blocks[bi + 1]
                        pre_qk = make_qk(nt_, nib)(0)
                    st = sts[jt]
                    eb = load_eb(t, ib, jt)
                    et0 = stream.tile([128, 2 * IB], f16, tag="et0", bufs=5)
                    nc.scalar.activation(et0[:], st[:], Exp)
                    et = stream.tile([128, 2 * IB], f16, tag="et", bufs=5)
                    nc.vector.tensor_mul(et[:], et0[:], eb)
                    if jt in (1, 2) and pending_norms:
                        # one half of the previous block's norms per slot,
                        # deferred + split so the DVE never sees a >1.5us
                        # burst between this block's et-multiplies
                        pending_norms.pop(0)()
                    for fn in tasks.get((bi, jt), ()):
                        fn()
                    if pend_av is not None:
                        pend_av()
                    pend_av = make_av(jt, et)
                if bi + 1 < len(blocks):
                    pend_av()
                else:
                    # last block: slot the nt=6,7 out-projection partials
                    # around the final AV so the PE tail window stays full
                    for fn in tail_pre[6][: KT - 1]:
                        fn()
                    pend_av()
                    for fn in tail_pre[7][: KT - 1]:
                        fn()

                def make_norm(po, ups, t=t, ib=ib):
                    def go():
                        rb = stream.tile(
                            [64, IB], f32, tag="rb", name=f"rb{t}{ib}{po}"
                        )
                        nc.vector.reciprocal_approx_fast(rb[:, :], ups[0:64, :])
                        nc.vector.tensor_mul(
                            Utn_sb[t][ib][po : po + 64, :],
                            ups[64:128, :],
                            rb[:, :],
                        )
                    return go

                pending_norms.append(make_norm(0, ups_e))
                pending_norms.append(make_norm(64, ups_o))
            flush_norms()

            # ---- Phase 3: final k-tile of each remaining output projection.
            # The k=3 matmul is split per head-half so the rows-0:64 halves
            # run right after nmul_e of the last norm, overlapping nmul_o;
            # the final DMAs alternate sync/scalar queues to halve the
            # serialized tail transfer time.
            for lo in (0, 64):
                for nt in (4, 5, 6, 7):
                    ps = out_ps_boxes[nt][0]
                    nc.tensor.matmul(
                        ps[:],
                        Utn_sb[KT - 1][1][lo : lo + 64, (nt % 4) * 128 : (nt % 4 + 1) * 128],
                        woV[lo : lo + 64, KT - 1, :],
                        start=False,
                        stop=(lo == 64),
                    )
            for nt in (4, 5, 6, 7):
                osb = opool.tile([128, D], f32, tag="osb", name=f"osb{nt}")
                psum_copy(osb[:], out_ps_boxes[nt][0][:])
                eng = nc.sync if nt % 2 == 0 else nc.scalar
                eng.dma_start(out[nt * 128 : (nt + 1) * 128, :], osb[:])

    return nc


def _get_graph():
    if "nc" not in _CACHE:
        nc = _build_graph()
        nc.compile()
        _CACHE["nc"] = nc
    return _CACHE["nc"]


def _prep_inputs(x, pos_bias, w_qkv, w_out):
    x = np.asarray(x, dtype=np.float32)
    pos_bias = np.asarray(pos_bias, dtype=np.float32)
    w_qkv = np.asarray(w_qkv, dtype=np.float32)
    w_out = np.asarray(w_out, dtype=np.float32)

    wqkv_mod = w_qkv.copy()
    wqkv_mod[:, :D] *= SCALE
    wout16 = w_out.astype(np.float16)
    wqkv16 = wqkv_mod.astype(np.float16)
    # prepacked exp(bias^T) tiles: ebt[t, ib, jt] = [128 j, he-i | ho-i]
    ebt = np.exp(pos_bias.transpose(0, 2, 1)).astype(np.float16)  # [h, j, i]
    ebt4 = ebt.reshape(KT, 2, NJT, 128, NIB, IB)  # [t, par, jt, p, ib, i]
    ebt_tiles = np.ascontiguousarray(
        ebt4.transpose(0, 4, 2, 3, 1, 5).reshape(KT, NIB, NJT, 128, 2 * IB)
    )

    in_maps = []
    for b in range(NCORES):
        in_maps.append(
            {
                "xT": np.ascontiguousarray(x[b].T.astype(np.float16)),
                "wqkv": wqkv16,
                "wout": wout16,
                "ebt": ebt_tiles,
            }
        )
    return in_maps


def _run(x, pos_bias, w_qkv, w_out, trace=False):
    from concourse.bass_utils import run_bass_kernel_spmd

    nc = _get_graph()
    in_maps = _prep_inputs(x, pos_bias, w_qkv, w_out)
    res = run_bass_kernel_spmd(
        nc, in_maps, core_ids=list(range(NCORES)), trace=trace
    )
    outs = np.stack([np.asarray(res.results[b]["out"]) for b in range(NCORES)])
    return outs.astype(np.float32), res


def kernel(x, pos_bias, w_qkv, w_out):
    outs, _ = _run(x, pos_bias, w_qkv, w_out, trace=False)
    return outs
